# revision 1
# baseline (speedup 1.0000x reference)
"""Causal single-head attention block for Trainium2, SPMD across 8 NeuronCores.

Problem (hardcoded):
    x:     [4, 2048, 1024] f32
    w_qkv: [1024, 3072]    f32   (q | k | v column blocks)
    w_out: [1024, 1024]    f32
    b_out: [1024]          f32
    y = softmax(causal(q @ k.T / 32)) @ v @ w_out + b_out     -> [4, 2048, 1024]

Sharding: 2 cores per batch element. Within a batch, the 16 query subtiles of
128 rows are dealt round-robin to the core pair (core parity h gets subtiles
s = 2k + h, k = 0..7) so both cores see the identical causal work profile
(key-chunk counts [1,1,2,2,3,3,4,4]) and a single SPMD program serves all 8
cores; per-core behavior differs only through input data (xT / gathered xQ /
mask tables). Each core computes K^T (SBUF-resident) and V (DRAM round-trip)
for the full 2048 keys of its batch, Q^T for its own 1024 queries, the
causal-masked softmax, attention-weighted values, and the output projection.

All matmuls run in float32r (TF32-like PE mode, 4x the fp32 matmul rate).
"""

import numpy as np

import concourse.mybir as mybir
import concourse.tile as tile
from concourse import bacc
from concourse.bass_utils import run_bass_kernel_spmd

FP32 = mybir.dt.float32
FP32R = mybir.dt.float32r
BF16 = mybir.dt.bfloat16
AF = mybir.ActivationFunctionType
ALU = mybir.AluOpType

B, S, D, NI, NO = 4, 2048, 1024, 1024, 1024
NCORES = 8
P = 128
DC = D // P    # 8 contraction chunks for the projections
IC = NI // P   # 8 inner-dim chunks
RC = S // 512  # 4 key/row production chunks
NSUB = 8       # local 128-row query subtiles per core
CC = [k // 2 + 1 for k in range(NSUB)]  # 512-key chunks per local subtile
SCALE = float(NI) ** -0.5
NEG = -1.0e9

_CACHED = {}


def _build():
    nc = bacc.Bacc(None, target_bir_lowering=False, debug=False, num_devices=NCORES)

    xT = nc.dram_tensor("xT", [D, S], FP32R, kind="ExternalInput").ap()
    xQ = nc.dram_tensor("xQ", [D, NSUB * P], FP32R, kind="ExternalInput").ap()
    wk_d = nc.dram_tensor("wk", [D, NI], FP32R, kind="ExternalInput").ap()
    wv_d = nc.dram_tensor("wv", [D, NI], FP32R, kind="ExternalInput").ap()
    wq_d = nc.dram_tensor("wq", [D, NI], FP32R, kind="ExternalInput").ap()
    wo_d = nc.dram_tensor("wo", [NI, NO], FP32R, kind="ExternalInput").ap()
    masks = nc.dram_tensor("masks", [NSUB, P, 512], BF16, kind="ExternalInput").ap()
    bb = nc.dram_tensor("bb", [P, NO], FP32, kind="ExternalInput").ap()
    ident = nc.dram_tensor("ident", [P, P], FP32R, kind="ExternalInput").ap()
    y = nc.dram_tensor("y", [NSUB * P, NO], FP32, kind="ExternalOutput").ap()

    with tile.TileContext(nc) as tc:
        with (
            tc.tile_pool(name="const", bufs=1) as constp,
            tc.tile_pool(name="ktpool", bufs=IC) as ktp,
            tc.tile_pool(name="qtpool", bufs=IC) as qtp,
            tc.tile_pool(name="accp", bufs=2, space="PSUM") as accp,
            tc.tile_pool(name="tpp", bufs=2, space="PSUM") as tpp,
            tc.tile_pool(name="opp", bufs=4, space="PSUM") as opp,
            tc.tile_pool(name="dram", bufs=1, space="DRAM") as dramp,
        ):
            ident_sb = constp.tile([P, P], FP32R, name="ident_sb", tag="id")
            nc.sync.dma_start(out=ident_sb[:], in_=ident[:])
            b_sb = constp.tile([P, NO], FP32, name="b_sb", tag="b")
            nc.sync.dma_start(out=b_sb[:], in_=bb[:])
            mask_sb = constp.tile([P, NSUB, 512], BF16, name="mask_sb", tag="mask")
            for k in range(NSUB):
                nc.sync.dma_start(out=mask_sb[:, k, :], in_=masks[k])

            KT = [ktp.tile([P, S], FP32R, name=f"kt{i}", tag="kt") for i in range(IC)]
            QT = [
                qtp.tile([P, NSUB * P], FP32R, name=f"qt{i}", tag="qt")
                for i in range(IC)
            ]
            v_dram = dramp.tile([S, NI], FP32R, name="v_dram", tag="vd")

            with tc.tile_pool(name="wpool", bufs=2 * DC) as wp:
                def load_w(src, label, nsplit):
                    ts = []
                    for d in range(DC):
                        t = wp.tile([P, NI], FP32R, name=f"{label}{d}", tag="w")
                        w_ = NI // nsplit
                        for q in range(nsplit):
                            nc.sync.dma_start(
                                out=t[:, w_ * q:w_ * (q + 1)],
                                in_=src[P * d:P * (d + 1), w_ * q:w_ * (q + 1)],
                            )
                        ts.append(t)
                    return ts

                with tc.tile_pool(name="xtp", bufs=12) as xtp:
                    def load_xt(rc):
                        ts = []
                        for d in range(DC):
                            t = xtp.tile([P, 512], FP32R, name=f"x{rc}_{d}", tag="xt")
                            for q in range(2):
                                nc.sync.dma_start(
                                    out=t[:, 256 * q:256 * (q + 1)],
                                    in_=xT[P * d:P * (d + 1),
                                           512 * rc + 256 * q:512 * rc + 256 * (q + 1)],
                                )
                            ts.append(t)
                        return ts

                    # ---- Phase 0: Q^T for all 1024 local queries ----
                    # wq/xq DMAs interleaved per d so the first psum's inputs
                    # arrive in consumption order.
                    wq = []
                    for qh in range(2):
                        xqs = []
                        for d in range(DC):
                            if qh == 0:
                                wt = wp.tile([P, NI], FP32R, name=f"wq{d}", tag="w")
                                for q in range(4):
                                    nc.sync.dma_start(
                                        out=wt[:, 256 * q:256 * (q + 1)],
                                        in_=wq_d[P * d:P * (d + 1),
                                                 256 * q:256 * (q + 1)],
                                    )
                                wq.append(wt)
                            t = xtp.tile([P, 512], FP32R, name=f"xq{qh}_{d}", tag="xt")
                            for q in range(2):
                                nc.sync.dma_start(
                                    out=t[:, 256 * q:256 * (q + 1)],
                                    in_=xQ[P * d:P * (d + 1),
                                           512 * qh + 256 * q:
                                           512 * qh + 256 * (q + 1)],
                                )
                            xqs.append(t)
                        if qh == 0:
                            wk = load_w(wk_d, "wk", 2)
                        for i in range(IC):
                            ps = accp.tile([P, 512], FP32, name="ps_qt", tag="acc")
                            for d in range(DC):
                                nc.tensor.matmul(
                                    ps[:], wq[d][:, P * i:P * (i + 1)], xqs[d][:],
                                    start=(d == 0), stop=(d == DC - 1),
                                )
                            nc.vector.tensor_copy(
                                QT[i][:, 512 * qh:512 * (qh + 1)], ps[:]
                            )
                    xt0 = load_xt(0)      # prefetch K/V chunk 0

                    # ---- Phase 1: K^T (SBUF-resident) and V (DRAM) ----
                    with tc.tile_pool(name="vst", bufs=4) as vstp:
                        wv = load_w(wv_d, "wv", 2)  # reuses wq's slots after Q^T
                        for rc in range(RC):
                            xts = xt0 if rc == 0 else load_xt(rc)
                            for i in range(IC):
                                ps = accp.tile([P, 512], FP32, name="ps_kt", tag="acc")
                                for d in range(DC):
                                    nc.tensor.matmul(
                                        ps[:], wk[d][:, P * i:P * (i + 1)], xts[d][:],
                                        start=(d == 0), stop=(d == DC - 1),
                                    )
                                nc.vector.tensor_copy(
                                    KT[i][:, 512 * rc:512 * (rc + 1)], ps[:]
                                )
                            for vs in range(4):
                                row = 512 * rc + P * vs
                                for ih in range(2):
                                    ps = accp.tile([P, 512], FP32, name="ps_v",
                                                   tag="acc")
                                    for d in range(DC):
                                        nc.tensor.matmul(
                                            ps[:],
                                            xts[d][:, P * vs:P * (vs + 1)],
                                            wv[d][:, 512 * ih:512 * (ih + 1)],
                                            start=(d == 0), stop=(d == DC - 1),
                                        )
                                    vt = vstp.tile([P, 512], FP32R, name="vstage",
                                                   tag="vst")
                                    nc.vector.tensor_copy(vt[:], ps[:])
                                    nc.sync.dma_start(
                                        out=v_dram[row:row + P,
                                                   512 * ih:512 * (ih + 1)],
                                        in_=vt[:],
                                    )

            # ---- attention, 4 pair-groups of 2 subtiles ----
            with tc.tile_pool(name="wopool", bufs=DC) as wop, \
                 tc.tile_pool(name="vfixp", bufs=4) as vfixp:
                wo = []
                for d in range(DC):
                    t = wop.tile([P, NI], FP32R, name=f"wo{d}", tag="wo")
                    for q in range(2):
                        nc.sync.dma_start(
                            out=t[:, 512 * q:512 * (q + 1)],
                            in_=wo_d[P * d:P * (d + 1), 512 * q:512 * (q + 1)],
                        )
                    wo.append(t)
                # V rows [0:512) are read by every group: pin them in SBUF
                vfix = []
                for t in range(4):
                    vf = vfixp.tile([P, NI], FP32R, name=f"vfix{t}", tag="vfix")
                    for q in range(2):
                        nc.sync.dma_start(
                            out=vf[:, 512 * q:512 * (q + 1)],
                            in_=v_dram[P * t:P * (t + 1), 512 * q:512 * (q + 1)],
                        )
                    vfix.append(vf)
                with (
                    tc.tile_pool(name="ppool", bufs=2) as ppool,
                    tc.tile_pool(name="ptpool", bufs=3) as ptpool,
                    tc.tile_pool(name="otpool", bufs=8) as otpool,
                    tc.tile_pool(name="vrd", bufs=4) as vrdp,
                    tc.tile_pool(name="ypool", bufs=2) as ypool,
                    tc.tile_pool(name="stp", bufs=4) as stp,
                ):
                    for g in range(4):
                        L = g + 1
                        k0, k1 = 2 * g, 2 * g + 1
                        Ps = {}
                        for k in (k0, k1):
                            p_t = ppool.tile([P, 4 * 512], FP32R, name=f"p{k}", tag="p")
                            sums = stp.tile([P, 4], FP32, name=f"sums{k}", tag="sums")
                            # diagonal chunk first: its mask+exp chain overlaps
                            # the remaining chunks' matmuls
                            for kc in ([L - 1] + list(range(L - 1))):
                                ps = accp.tile([P, 512], FP32, name="ps_sim", tag="acc")
                                for i in range(IC):
                                    nc.tensor.matmul(
                                        ps[:],
                                        QT[i][:, P * k:P * (k + 1)],
                                        KT[i][:, 512 * kc:512 * (kc + 1)],
                                        start=(i == 0), stop=(i == IC - 1),
                                    )
                                if kc == L - 1:
                                    nc.vector.tensor_tensor(
                                        out=ps[:], in0=ps[:], in1=mask_sb[:, k, :],
                                        op=ALU.add,
                                    )
                                nc.scalar.activation(
                                    p_t[:, 512 * kc:512 * (kc + 1)], ps[:], AF.Exp,
                                    scale=SCALE, accum_out=sums[:, kc:kc + 1],
                                )
                            ssum = stp.tile([P, 1], FP32, name=f"ssum{k}", tag="ss")
                            nc.vector.tensor_reduce(
                                ssum[:], sums[:, :L], axis=mybir.AxisListType.X,
                                op=ALU.add,
                            )
                            rsum = stp.tile([P, 1], FP32, name=f"rsum{k}", tag="rs")
                            nc.vector.reciprocal(rsum[:], ssum[:])
                            nc.vector.tensor_scalar_mul(
                                p_t[:, :512 * L], p_t[:, :512 * L], rsum[:]
                            )
                            Ps[k] = p_t

                        ops = [
                            opp.tile([P, 512], FP32, name=f"op{g}_{j}", tag="op")
                            for j in range(4)
                        ]
                        nt = 4 * L
                        for t in range(nt):
                            tp_ps = tpp.tile([P, 256], FP32R, name="tp", tag="tp")
                            nc.tensor.transpose(
                                tp_ps[:, 0:P], Ps[k0][:, P * t:P * (t + 1)], ident_sb[:]
                            )
                            nc.tensor.transpose(
                                tp_ps[:, P:256], Ps[k1][:, P * t:P * (t + 1)],
                                ident_sb[:]
                            )
                            pt_t = ptpool.tile([P, 256], FP32R, name="pt", tag="pt")
                            nc.vector.tensor_copy(pt_t[:], tp_ps[:])
                            if t < 4:
                                v_t = vfix[t]
                            else:
                                v_t = vrdp.tile([P, NI], FP32R, name="v_t", tag="v")
                                for q in range(2):
                                    nc.sync.dma_start(
                                        out=v_t[:, 512 * q:512 * (q + 1)],
                                        in_=v_dram[P * t:P * (t + 1),
                                                   512 * q:512 * (q + 1)],
                                    )
                            for m in range(IC):
                                # one accumulation group per PSUM bank: start
                                # only on the bank's first matmul (whole-bank
                                # pending-zero makes the sibling column-half's
                                # first write an overwrite), stop on its last
                                nc.tensor.matmul(
                                    ops[m // 2][:, 256 * (m % 2):256 * (m % 2) + 256],
                                    v_t[:, P * m:P * (m + 1)],
                                    pt_t[:],
                                    start=(t == 0 and m % 2 == 0),
                                    stop=(t == nt - 1 and m % 2 == 1),
                                )

                        oT = []
                        for m in range(IC):
                            ot = otpool.tile([P, 256], FP32R, name=f"ot{g}_{m}",
                                             tag="ot")
                            nc.vector.tensor_copy(
                                ot[:], ops[m // 2][:, 256 * (m % 2):256 * (m % 2) + 256]
                            )
                            oT.append(ot)

                        # ---- output projection for this group's 2 subtiles ----
                        # y psums cycle through the opp pool so accp stays free
                        # for the next group's sim matmuls
                        for col, k in enumerate((k0, k1)):
                            for oh in range(2):
                                ps = opp.tile([P, 512], FP32, name="ps_y", tag="op")
                                for i in range(IC):
                                    nc.tensor.matmul(
                                        ps[:],
                                        oT[i][:, P * col:P * (col + 1)],
                                        wo[i][:, 512 * oh:512 * (oh + 1)],
                                        start=(i == 0), stop=(i == IC - 1),
                                    )
                                y_sb = ypool.tile([P, 512], FP32, name="y_sb", tag="y")
                                nc.vector.tensor_tensor(
                                    out=y_sb[:], in0=ps[:],
                                    in1=b_sb[:, 512 * oh:512 * (oh + 1)], op=ALU.add,
                                )
                                nc.sync.dma_start(
                                    out=y[P * k:P * (k + 1), 512 * oh:512 * (oh + 1)],
                                    in_=y_sb[:],
                                )

    nc.compile()
    return nc


def _prep_inputs(x, w_qkv, w_out, b_out):
    import ml_dtypes
    x = np.asarray(x, dtype=np.float32)
    w_qkv = np.asarray(w_qkv, dtype=np.float32)
    w_out = np.asarray(w_out, dtype=np.float32)
    b_out = np.asarray(b_out, dtype=np.float32)

    wq = np.ascontiguousarray(w_qkv[:, 0 * NI:1 * NI])
    wk = np.ascontiguousarray(w_qkv[:, 1 * NI:2 * NI])
    wv = np.ascontiguousarray(w_qkv[:, 2 * NI:3 * NI])
    b_bcast = np.ascontiguousarray(np.broadcast_to(b_out[None, :], (P, NO)))
    ident = np.eye(P, dtype=np.float32)

    xTs = [np.ascontiguousarray(x[b].T) for b in range(B)]

    in_maps = []
    for c in range(NCORES):
        b, h = c // 2, c % 2
        subs = [2 * k + h for k in range(NSUB)]
        xQ = np.concatenate(
            [xTs[b][:, P * s:P * (s + 1)] for s in subs], axis=1
        )
        m = np.empty((NSUB, P, 512), dtype=ml_dtypes.bfloat16)
        cpos = np.arange(512)[None, :]
        prow = np.arange(P)[:, None]
        for k in range(NSUB):
            off = P * subs[k] - 512 * (CC[k] - 1)
            m[k] = np.where(cpos <= off + prow, 0.0, NEG)
        in_maps.append({
            "xT": xTs[b], "xQ": np.ascontiguousarray(xQ),
            "wk": wk, "wv": wv, "wq": wq, "wo": w_out,
            "masks": m, "bb": b_bcast, "ident": ident,
        })
    return in_maps


def _run(x, w_qkv, w_out, b_out, trace=False, **kw):
    if "nc" not in _CACHED:
        _CACHED["nc"] = _build()
    nc = _CACHED["nc"]
    in_maps = _prep_inputs(x, w_qkv, w_out, b_out)
    res = run_bass_kernel_spmd(nc, in_maps, list(range(NCORES)), trace=trace, **kw)
    out = np.empty((B, S, NO), dtype=np.float32)
    for c in range(NCORES):
        b, h = c // 2, c % 2
        yc = res.results[c]["y"]
        for k in range(NSUB):
            s = 2 * k + h
            out[b, P * s:P * (s + 1), :] = yc[P * k:P * (k + 1), :]
    return out, res


def kernel(x, w_qkv, w_out, b_out):
    out, _ = _run(x, w_qkv, w_out, b_out, trace=False)
    return out



# revision 9
# speedup vs baseline: 1.6339x; 1.6339x over previous
"""Causal single-head attention block for Trainium2, SPMD across 8 NeuronCores.

Problem (hardcoded):
    x:     [4, 2048, 1024] f32
    w_qkv: [1024, 3072]    f32   (q | k | v column blocks)
    w_out: [1024, 1024]    f32
    b_out: [1024]          f32
    y = softmax(causal(q @ k.T / 32)) @ v @ w_out + b_out     -> [4, 2048, 1024]

Sharding: 2 cores per batch element. Within a batch, the 16 query subtiles of
128 rows are dealt round-robin to the core pair (core parity h gets subtiles
s = 2k + h, k = 0..7) so both cores see the identical causal work profile
(key-chunk counts [1,1,2,2,3,3,4,4]) and a single SPMD program serves all 8
cores; per-core behavior differs only through input data.

All matmul operands are bf16 (SBUF); accumulation is f32 in PSUM. bf16
weights enable the compiler's fast-weight-load path so LDWEIGHTS hides under
the matmuls. K^T, Q^T and V all stay SBUF-resident for the whole kernel (no
DRAM round-trip). Softmax normalization is postponed: unnormalized exp(sim)
feeds attn@V and the 1/rowsum is applied on the output-projection PSUM
(queries are on partitions there), so nothing serializes before the
transposes.
"""

import numpy as np

import concourse.mybir as mybir
import concourse.tile as tile
from concourse import bacc
from concourse.bass_utils import run_bass_kernel_spmd

FP32 = mybir.dt.float32
BF16 = mybir.dt.bfloat16
AF = mybir.ActivationFunctionType
ALU = mybir.AluOpType

B, S, D, NI, NO = 4, 2048, 1024, 1024, 1024
NCORES = 8
P = 128
DC = D // P    # 8 contraction chunks for the projections
IC = NI // P   # 8 inner-dim chunks
RC = S // 512  # 4 key production chunks
NT = S // P    # 16 key tiles
NSUB = 8       # local 128-row query subtiles per core
CC = [k // 2 + 1 for k in range(NSUB)]  # 512-key chunks per local subtile
SCALE = float(NI) ** -0.5
NEG = -1.0e9

_CACHED = {}


def _build():
    nc = bacc.Bacc(None, target_bir_lowering=False, debug=False, num_devices=NCORES)

    xT = nc.dram_tensor("xT", [D, S], BF16, kind="ExternalInput").ap()
    xQ = nc.dram_tensor("xQ", [D, NSUB * P], BF16, kind="ExternalInput").ap()
    wk_d = nc.dram_tensor("wk", [D, NI], BF16, kind="ExternalInput").ap()
    wv_d = nc.dram_tensor("wv", [D, NI], BF16, kind="ExternalInput").ap()
    wq_d = nc.dram_tensor("wq", [D, NI], BF16, kind="ExternalInput").ap()
    wo_d = nc.dram_tensor("wo", [NI, NO], BF16, kind="ExternalInput").ap()
    masks = nc.dram_tensor("masks", [P, 2 * 512], BF16, kind="ExternalInput").ap()
    bb = nc.dram_tensor("bb", [P, NO], BF16, kind="ExternalInput").ap()
    ident = nc.dram_tensor("ident", [P, P], BF16, kind="ExternalInput").ap()
    y = nc.dram_tensor("y", [NSUB * P, NO], FP32, kind="ExternalOutput").ap()

    with tile.TileContext(nc) as tc:
        with (
            tc.tile_pool(name="const", bufs=1) as constp,
            tc.tile_pool(name="ktpool", bufs=IC) as ktp,
            tc.tile_pool(name="qtpool", bufs=IC) as qtp,
            tc.tile_pool(name="vpool", bufs=NT) as vpool,
            tc.tile_pool(name="wopool", bufs=DC) as wop,
            tc.tile_pool(name="accp", bufs=2, space="PSUM") as accp,
        ):
            KT = [ktp.tile([P, S], BF16, name=f"kt{i}", tag="kt") for i in range(IC)]
            QT = [
                qtp.tile([P, NSUB * P], BF16, name=f"qt{i}", tag="qt")
                for i in range(IC)
            ]
            V = [vpool.tile([P, NI], BF16, name=f"v{t}", tag="v") for t in range(NT)]

            with (
                tc.tile_pool(name="xfp", bufs=RC * DC) as xfp,
                tc.tile_pool(name="wkp", bufs=DC) as wkp,
                tc.tile_pool(name="wvp", bufs=DC) as wvp,
                tc.tile_pool(name="wqp", bufs=DC) as wqp,
                tc.tile_pool(name="xqp", bufs=DC) as xqp,
            ):
                # ---- all input DMAs, emitted in consumption priority order ----
                # xT loads are chunked rc-major so the first K^T pass is
                # gated on 3MB (wk + rc0) instead of 6MB
                wk = []
                xfc = [[None] * DC for _ in range(RC)]
                for d in range(DC):
                    t = wkp.tile([P, NI], BF16, name=f"wk{d}", tag="wk")
                    nc.sync.dma_start(out=t[:], in_=wk_d[P * d:P * (d + 1), :])
                    wk.append(t)
                    t = xfp.tile([P, 512], BF16, name=f"xf0_{d}", tag="xf")
                    nc.sync.dma_start(out=t[:], in_=xT[P * d:P * (d + 1), 0:512])
                    xfc[0][d] = t
                for rc in range(1, RC):
                    for d in range(DC):
                        t = xfp.tile([P, 512], BF16, name=f"xf{rc}_{d}", tag="xf")
                        nc.sync.dma_start(
                            out=t[:],
                            in_=xT[P * d:P * (d + 1), 512 * rc:512 * (rc + 1)],
                        )
                        xfc[rc][d] = t
                wv = []
                for d in range(DC):
                    t = wvp.tile([P, NI], BF16, name=f"wv{d}", tag="wv")
                    nc.sync.dma_start(out=t[:], in_=wv_d[P * d:P * (d + 1), :])
                    wv.append(t)
                wq = []
                xq = []
                for d in range(DC):
                    t = wqp.tile([P, NI], BF16, name=f"wq{d}", tag="wq")
                    nc.sync.dma_start(out=t[:], in_=wq_d[P * d:P * (d + 1), :])
                    wq.append(t)
                    t = xqp.tile([P, NSUB * P], BF16, name=f"xq{d}", tag="xq")
                    nc.sync.dma_start(out=t[:], in_=xQ[P * d:P * (d + 1), :])
                    xq.append(t)
                wo = []
                for d in range(DC):
                    t = wop.tile([P, NO], BF16, name=f"wo{d}", tag="wo")
                    nc.sync.dma_start(out=t[:], in_=wo_d[P * d:P * (d + 1), :])
                    wo.append(t)
                ident_sb = constp.tile([P, P], BF16, name="ident_sb", tag="id")
                nc.sync.dma_start(out=ident_sb[:], in_=ident[:])
                b_sb = constp.tile([P, NO], BF16, name="b_sb", tag="b")
                nc.sync.dma_start(out=b_sb[:], in_=bb[:])
                mask_sb = constp.tile([P, 2 * 512], BF16, name="mask_sb", tag="mask")
                nc.sync.dma_start(out=mask_sb[:], in_=masks[:])

                # ---- Phase 1: K^T (SBUF-resident) ----
                for rc in range(RC):
                    for i in range(IC):
                        ps = accp.tile([P, 512], FP32, name="ps_kt", tag="acc")
                        for d in range(DC):
                            nc.tensor.matmul(
                                ps[:], wk[d][:, P * i:P * (i + 1)],
                                xfc[rc][d][:],
                                start=(d == 0), stop=(d == DC - 1),
                            )
                        eng = nc.vector if i % 2 == 0 else nc.scalar
                        if i % 2 == 0:
                            eng.tensor_copy(
                                KT[i][:, 512 * rc:512 * (rc + 1)], ps[:]
                            )
                        else:
                            eng.activation(
                                KT[i][:, 512 * rc:512 * (rc + 1)], ps[:], AF.Copy
                            )

                # ---- Phase 2: V (SBUF-resident, natural [keys, inner]) ----
                for vs in range(NT):
                    for ih in range(2):
                        ps = accp.tile([P, 512], FP32, name="ps_v", tag="acc")
                        for d in range(DC):
                            co = P * (vs % 4)
                            nc.tensor.matmul(
                                ps[:],
                                xfc[vs // 4][d][:, co:co + P],
                                wv[d][:, 512 * ih:512 * (ih + 1)],
                                start=(d == 0), stop=(d == DC - 1),
                            )
                        if (2 * vs + ih) % 2 == 0:
                            nc.vector.tensor_copy(
                                V[vs][:, 512 * ih:512 * (ih + 1)], ps[:]
                            )
                        else:
                            nc.scalar.activation(
                                V[vs][:, 512 * ih:512 * (ih + 1)], ps[:], AF.Copy
                            )

                # ---- Phase 3: Q^T for the local 1024 queries ----
                for qh in range(2):
                    for i in range(IC):
                        ps = accp.tile([P, 512], FP32, name="ps_qt", tag="acc")
                        for d in range(DC):
                            nc.tensor.matmul(
                                ps[:], wq[d][:, P * i:P * (i + 1)],
                                xq[d][:, 512 * qh:512 * (qh + 1)],
                                start=(d == 0), stop=(d == DC - 1),
                            )
                        if i % 2 == 0:
                            nc.vector.tensor_copy(
                                QT[i][:, 512 * qh:512 * (qh + 1)], ps[:]
                            )
                        else:
                            nc.scalar.activation(
                                QT[i][:, 512 * qh:512 * (qh + 1)], ps[:], AF.Copy
                            )

            # ---- attention, 4 pair-groups of 2 subtiles ----
            with (
                tc.tile_pool(name="ppool", bufs=3) as ppool,
                tc.tile_pool(name="ptpool", bufs=3) as ptpool,
                tc.tile_pool(name="otpool", bufs=8) as otpool,
                tc.tile_pool(name="ypool", bufs=4) as ypool,
                tc.tile_pool(name="stp", bufs=12) as stp,
                tc.tile_pool(name="tpp", bufs=2, space="PSUM") as tpp,
                tc.tile_pool(name="opp", bufs=4, space="PSUM") as opp,
            ):
                for g in range(4):
                    L = g + 1
                    k0, k1 = 2 * g, 2 * g + 1
                    Ps = {}
                    Rs = {}
                    for k in (k0, k1):
                        p_t = ppool.tile([P, 4 * 512], BF16, name=f"p{k}", tag="p")
                        sums = stp.tile([P, 4], FP32, name=f"sums{k}", tag="sums")
                        # diagonal chunk first: its mask+exp chain overlaps
                        # the remaining chunks' matmuls
                        for kc in ([L - 1] + list(range(L - 1))):
                            ps = accp.tile([P, 512], FP32, name="ps_sim", tag="acc")
                            for i in range(IC):
                                nc.tensor.matmul(
                                    ps[:],
                                    QT[i][:, P * k:P * (k + 1)],
                                    KT[i][:, 512 * kc:512 * (kc + 1)],
                                    start=(i == 0), stop=(i == IC - 1),
                                )
                            if kc == L - 1:
                                mo = 512 * (k % 2)
                                nc.vector.tensor_tensor(
                                    out=ps[:], in0=ps[:],
                                    in1=mask_sb[:, mo:mo + 512], op=ALU.add,
                                )
                            nc.scalar.activation(
                                p_t[:, 512 * kc:512 * (kc + 1)], ps[:], AF.Exp,
                                scale=SCALE, accum_out=sums[:, kc:kc + 1],
                            )
                        rsum = stp.tile([P, 1], FP32, name=f"rsum{k}", tag="rs")
                        if L == 1:
                            nc.vector.reciprocal(rsum[:], sums[:, 0:1])
                        else:
                            ssum = stp.tile([P, 1], FP32, name=f"ssum{k}", tag="ss")
                            nc.vector.tensor_reduce(
                                ssum[:], sums[:, :L], axis=mybir.AxisListType.X,
                                op=ALU.add,
                            )
                            nc.vector.reciprocal(rsum[:], ssum[:])
                        Ps[k] = p_t
                        Rs[k] = rsum

                    ops = [
                        opp.tile([P, 512], FP32, name=f"op{g}_{j}", tag="op")
                        for j in range(4)
                    ]
                    nt = 4 * L

                    def transpose_pair(t):
                        tp_ps = tpp.tile([P, 256], BF16, name="tp", tag="tp")
                        nc.tensor.transpose(
                            tp_ps[:, 0:P], Ps[k0][:, P * t:P * (t + 1)], ident_sb[:]
                        )
                        nc.tensor.transpose(
                            tp_ps[:, P:256], Ps[k1][:, P * t:P * (t + 1)],
                            ident_sb[:]
                        )
                        pt_t = ptpool.tile([P, 256], BF16, name="pt", tag="pt")
                        nc.vector.tensor_copy(pt_t[:], tp_ps[:])
                        return pt_t

                    def attnv(t, pt_t):
                        for m in range(IC):
                            # one accumulation group per PSUM bank: start
                            # only on the bank's first matmul (whole-bank
                            # pending-zero makes the sibling column-half's
                            # first write an overwrite), stop on its last
                            nc.tensor.matmul(
                                ops[m // 2][:, 256 * (m % 2):256 * (m % 2) + 256],
                                V[t][:, P * m:P * (m + 1)],
                                pt_t[:],
                                start=(t == 0 and m % 2 == 0),
                                stop=(t == nt - 1 and m % 2 == 1),
                            )

                    # transposes run one iteration ahead of attn@V so the
                    # PSUM->SBUF copy latency hides under the matmuls
                    prev = transpose_pair(0)
                    for t in range(1, nt):
                        cur = transpose_pair(t)
                        attnv(t - 1, prev)
                        prev = cur
                    attnv(nt - 1, prev)

                    oT = []
                    for m in range(IC):
                        ot = otpool.tile([P, 256], BF16, name=f"ot{g}_{m}",
                                         tag="ot")
                        src = ops[m // 2][:, 256 * (m % 2):256 * (m % 2) + 256]
                        if m % 2 == 0:
                            nc.vector.tensor_copy(ot[:], src)
                        else:
                            nc.scalar.activation(ot[:], src, AF.Copy)
                        oT.append(ot)

                    # ---- output projection for this group's 2 subtiles ----
                    # y = (oT.T @ wo) * (1/rowsum) + b; the rowsum scale rides
                    # the scalar-engine PSUM drain (queries are on partitions)
                    for col, k in enumerate((k0, k1)):
                        for oh in range(2):
                            ps = opp.tile([P, 512], FP32, name="ps_y", tag="op")
                            for i in range(IC):
                                nc.tensor.matmul(
                                    ps[:],
                                    oT[i][:, P * col:P * (col + 1)],
                                    wo[i][:, 512 * oh:512 * (oh + 1)],
                                    start=(i == 0), stop=(i == IC - 1),
                                )
                            y_sb = ypool.tile([P, 512], FP32, name="y_sb", tag="y")
                            # the very last drain runs in two halves so the
                            # scale/bias/DMA chain pipelines at kernel end
                            halves = 2 if (g == 3 and col == 1 and oh == 1) else 1
                            hw_ = 512 // halves
                            for hh in range(halves):
                                sl = slice(hw_ * hh, hw_ * (hh + 1))
                                nc.scalar.activation(
                                    y_sb[:, sl], ps[:, sl], AF.Copy, scale=Rs[k][:]
                                )
                                nc.vector.tensor_tensor(
                                    out=y_sb[:, sl], in0=y_sb[:, sl],
                                    in1=b_sb[:, 512 * oh + hw_ * hh:
                                             512 * oh + hw_ * (hh + 1)],
                                    op=ALU.add,
                                )
                                nc.sync.dma_start(
                                    out=y[P * k:P * (k + 1),
                                          512 * oh + hw_ * hh:
                                          512 * oh + hw_ * (hh + 1)],
                                    in_=y_sb[:, sl],
                                )

    nc.compile()
    return nc


def _prep_inputs(x, w_qkv, w_out, b_out):
    import ml_dtypes
    bf = ml_dtypes.bfloat16
    x = np.asarray(x, dtype=np.float32)
    w_qkv = np.asarray(w_qkv, dtype=np.float32)

    wq = np.ascontiguousarray(w_qkv[:, 0 * NI:1 * NI].astype(bf))
    wk = np.ascontiguousarray(w_qkv[:, 1 * NI:2 * NI].astype(bf))
    wv = np.ascontiguousarray(w_qkv[:, 2 * NI:3 * NI].astype(bf))
    wo = np.ascontiguousarray(np.asarray(w_out, dtype=np.float32).astype(bf))
    b_bcast = np.ascontiguousarray(
        np.broadcast_to(np.asarray(b_out, dtype=np.float32)[None, :], (P, NO))
    ).astype(bf)
    ident = np.eye(P, dtype=np.float32).astype(bf)

    xTs = [np.ascontiguousarray(x[b].T.astype(bf)) for b in range(B)]

    in_maps = []
    cpos = np.arange(512)[None, :]
    prow = np.arange(P)[:, None]
    for c in range(NCORES):
        b, h = c // 2, c % 2
        subs = [2 * k + h for k in range(NSUB)]
        xQ = np.ascontiguousarray(np.concatenate(
            [xTs[b][:, P * s:P * (s + 1)] for s in subs], axis=1
        ))
        # two distinct diagonal masks: even local subtiles sit at chunk
        # offset 128h, odd ones at 256 + 128h
        m = np.empty((P, 2 * 512), dtype=bf)
        for par in range(2):
            off = 128 * h + 256 * par
            m[:, 512 * par:512 * (par + 1)] = np.where(
                cpos <= off + prow, 0.0, NEG
            )
        in_maps.append({
            "xT": xTs[b], "xQ": xQ,
            "wk": wk, "wv": wv, "wq": wq, "wo": wo,
            "masks": m, "bb": b_bcast, "ident": ident,
        })
    return in_maps


def _run(x, w_qkv, w_out, b_out, trace=False, **kw):
    if "nc" not in _CACHED:
        _CACHED["nc"] = _build()
    nc = _CACHED["nc"]
    in_maps = _prep_inputs(x, w_qkv, w_out, b_out)
    res = run_bass_kernel_spmd(nc, in_maps, list(range(NCORES)), trace=trace, **kw)
    out = np.empty((B, S, NO), dtype=np.float32)
    for c in range(NCORES):
        b, h = c // 2, c % 2
        yc = res.results[c]["y"]
        for k in range(NSUB):
            s = 2 * k + h
            out[b, P * s:P * (s + 1), :] = yc[P * k:P * (k + 1), :]
    return out, res


def kernel(x, w_qkv, w_out, b_out):
    out, _ = _run(x, w_qkv, w_out, b_out, trace=False)
    return out


# revision 10
# speedup vs baseline: 2.1293x; 1.3032x over previous
"""Causal single-head attention block for Trainium2, SPMD across 8 NeuronCores.

Problem (hardcoded):
    x:     [4, 2048, 1024] f32
    w_qkv: [1024, 3072]    f32   (q | k | v column blocks)
    w_out: [1024, 1024]    f32
    b_out: [1024]          f32
    y = softmax(causal(q @ k.T / 32)) @ v @ w_out + b_out     -> [4, 2048, 1024]

Sharding: 2 cores per batch element. Within a batch, the 16 query subtiles of
128 rows are dealt round-robin to the core pair (core parity h gets subtiles
s = 2k + h, k = 0..7) so both cores see the identical causal work profile
(key-chunk counts [1,1,2,2,3,3,4,4]) and a single SPMD program serves all 8
cores; per-core behavior differs only through input data.

Algebraic restructure (kills K and V production entirely):
    sim = (xQ^T Wq)(Wk^T x^T) = xQ^T M x^T,   M = Wq Wk^T  (host-precomputed)
        -> AT = M^T xQ on-device (cost of the old Q^T pass), then sim runs
           directly against the resident x^T tiles.
    O   = P (x Wv) = (x^T P^T)^T Wv
        -> Z = x^T P^T accumulates against natural-layout x tiles (cost of
           the old attn@V pass), then one small Wv^T Z pass per pair-group.

All matmul operands are bf16 (SBUF); accumulation is f32 in PSUM. bf16
weights take the fast-weight-load path so LDWEIGHTS hides under the matmuls.
Softmax normalization is postponed: unnormalized exp(sim) feeds Z and the
1/rowsum rides the output-projection PSUM drain (queries on partitions).
"""

import numpy as np

import concourse.mybir as mybir
import concourse.tile as tile
from concourse import bacc
from concourse.bass_utils import run_bass_kernel_spmd

FP32 = mybir.dt.float32
BF16 = mybir.dt.bfloat16
AF = mybir.ActivationFunctionType
ALU = mybir.AluOpType

B, S, D, NI, NO = 4, 2048, 1024, 1024, 1024
NCORES = 8
P = 128
DC = D // P    # 8 contraction chunks over the model dim
RC = S // 512  # 4 key chunks of 512
NT = S // P    # 16 key tiles of 128
NSUB = 8       # local 128-row query subtiles per core
CC = [k // 2 + 1 for k in range(NSUB)]  # 512-key chunks per local subtile
SCALE = float(NI) ** -0.5
NEG = -1.0e9

_CACHED = {}


def _build():
    nc = bacc.Bacc(None, target_bir_lowering=False, debug=False, num_devices=NCORES)

    xT = nc.dram_tensor("xT", [D, S], BF16, kind="ExternalInput").ap()
    xN_d = nc.dram_tensor("xN", [S, D], BF16, kind="ExternalInput").ap()
    xQ = nc.dram_tensor("xQ", [D, NSUB * P], BF16, kind="ExternalInput").ap()
    m_d = nc.dram_tensor("m", [D, D], BF16, kind="ExternalInput").ap()
    wv_d = nc.dram_tensor("wv", [D, NI], BF16, kind="ExternalInput").ap()
    wo_d = nc.dram_tensor("wo", [NI, NO], BF16, kind="ExternalInput").ap()
    masks = nc.dram_tensor("masks", [P, 2 * 512], BF16, kind="ExternalInput").ap()
    bb = nc.dram_tensor("bb", [P, NO], BF16, kind="ExternalInput").ap()
    ident = nc.dram_tensor("ident", [P, P], BF16, kind="ExternalInput").ap()
    y = nc.dram_tensor("y", [NSUB * P, NO], FP32, kind="ExternalOutput").ap()

    with tile.TileContext(nc) as tc:
        with (
            tc.tile_pool(name="const", bufs=1) as constp,
            tc.tile_pool(name="atpool", bufs=DC) as atp,
            tc.tile_pool(name="xfp", bufs=RC * DC) as xfp,
            tc.tile_pool(name="xnp", bufs=NT) as xnp,
            tc.tile_pool(name="wvpool", bufs=DC) as wvp,
            tc.tile_pool(name="wopool", bufs=DC) as wop,
            tc.tile_pool(name="accp", bufs=2, space="PSUM") as accp,
        ):
            AT = [atp.tile([P, NSUB * P], BF16, name=f"at{i}", tag="at")
                  for i in range(DC)]

            with (
                tc.tile_pool(name="mp", bufs=DC) as mp,
                tc.tile_pool(name="xqp", bufs=DC) as xqp,
            ):
                # ---- all input DMAs, emitted in consumption priority order ----
                m_t = []
                xq = []
                for d in range(DC):
                    t = mp.tile([P, D], BF16, name=f"m{d}", tag="m")
                    nc.sync.dma_start(out=t[:], in_=m_d[P * d:P * (d + 1), :])
                    m_t.append(t)
                    t = xqp.tile([P, NSUB * P], BF16, name=f"xq{d}", tag="xq")
                    nc.sync.dma_start(out=t[:], in_=xQ[P * d:P * (d + 1), :])
                    xq.append(t)
                xfc = [[None] * DC for _ in range(RC)]
                for rc in range(RC):
                    for d in range(DC):
                        t = xfp.tile([P, 512], BF16, name=f"xf{rc}_{d}", tag="xf")
                        nc.sync.dma_start(
                            out=t[:],
                            in_=xT[P * d:P * (d + 1), 512 * rc:512 * (rc + 1)],
                        )
                        xfc[rc][d] = t
                xN = []
                for t_ in range(NT):
                    t = xnp.tile([P, D], BF16, name=f"xn{t_}", tag="xn")
                    nc.sync.dma_start(out=t[:], in_=xN_d[P * t_:P * (t_ + 1), :])
                    xN.append(t)
                wv = []
                for d in range(DC):
                    t = wvp.tile([P, NI], BF16, name=f"wv{d}", tag="wv")
                    nc.sync.dma_start(out=t[:], in_=wv_d[P * d:P * (d + 1), :])
                    wv.append(t)
                wo = []
                for d in range(DC):
                    t = wop.tile([P, NO], BF16, name=f"wo{d}", tag="wo")
                    nc.sync.dma_start(out=t[:], in_=wo_d[P * d:P * (d + 1), :])
                    wo.append(t)
                ident_sb = constp.tile([P, P], BF16, name="ident_sb", tag="id")
                nc.sync.dma_start(out=ident_sb[:], in_=ident[:])
                b_sb = constp.tile([P, NO], BF16, name="b_sb", tag="b")
                nc.sync.dma_start(out=b_sb[:], in_=bb[:])
                mask_sb = constp.tile([P, 2 * 512], BF16, name="mask_sb", tag="mask")
                nc.sync.dma_start(out=mask_sb[:], in_=masks[:])

                # ---- Phase 1: AT = M^T xQ for the local 1024 queries ----
                for qh in range(2):
                    for i in range(DC):
                        ps = accp.tile([P, 512], FP32, name="ps_at", tag="acc")
                        for d in range(DC):
                            nc.tensor.matmul(
                                ps[:], m_t[d][:, P * i:P * (i + 1)],
                                xq[d][:, 512 * qh:512 * (qh + 1)],
                                start=(d == 0), stop=(d == DC - 1),
                            )
                        if i % 2 == 0:
                            nc.vector.tensor_copy(
                                AT[i][:, 512 * qh:512 * (qh + 1)], ps[:]
                            )
                        else:
                            nc.scalar.activation(
                                AT[i][:, 512 * qh:512 * (qh + 1)], ps[:], AF.Copy
                            )

            # ---- attention, 4 pair-groups of 2 subtiles ----
            with (
                tc.tile_pool(name="ppool", bufs=3) as ppool,
                tc.tile_pool(name="ptpool", bufs=8) as ptpool,
                tc.tile_pool(name="zpool", bufs=8) as zpool,
                tc.tile_pool(name="otpool", bufs=8) as otpool,
                tc.tile_pool(name="ypool", bufs=4) as ypool,
                tc.tile_pool(name="stp", bufs=12) as stp,
                tc.tile_pool(name="tpp", bufs=2, space="PSUM") as tpp,
                tc.tile_pool(name="opp", bufs=4, space="PSUM") as opp,
            ):
                for g in range(4):
                    L = g + 1
                    k0, k1 = 2 * g, 2 * g + 1
                    nt = 4 * L
                    Ps = {}
                    Rs = {}
                    for k in (k0, k1):
                        p_t = ppool.tile([P, 4 * 512], BF16, name=f"p{k}", tag="p")
                        sums = stp.tile([P, 4], FP32, name=f"sums{k}", tag="sums")
                        # ascending chunk order (diagonal last) lines up with
                        # the upfront transpose sweep that follows
                        for kc in range(L):
                            diag = kc == L - 1
                            # even subtiles only attend 256 into their
                            # diagonal chunk; the rest is masked anyway
                            w = 256 if (diag and k % 2 == 0) else 512
                            ps = accp.tile([P, 512], FP32, name="ps_sim", tag="acc")
                            for d in range(DC):
                                nc.tensor.matmul(
                                    ps[:, :w],
                                    AT[d][:, P * k:P * (k + 1)],
                                    xfc[kc][d][:, :w],
                                    start=(d == 0), stop=(d == DC - 1),
                                )
                            if diag:
                                mo = 512 * (k % 2)
                                nc.vector.tensor_tensor(
                                    out=ps[:, :w], in0=ps[:, :w],
                                    in1=mask_sb[:, mo:mo + w], op=ALU.add,
                                )
                            nc.scalar.activation(
                                p_t[:, 512 * kc:512 * kc + w], ps[:, :w], AF.Exp,
                                scale=SCALE, accum_out=sums[:, kc:kc + 1],
                            )
                            if w == 256:
                                nc.gpsimd.memset(
                                    p_t[:, 512 * kc + 256:512 * (kc + 1)], 0.0
                                )
                        rsum = stp.tile([P, 1], FP32, name=f"rsum{k}", tag="rs")
                        if L == 1:
                            nc.vector.reciprocal(rsum[:], sums[:, 0:1])
                        else:
                            ssum = stp.tile([P, 1], FP32, name=f"ssum{k}", tag="ss")
                            nc.vector.tensor_reduce(
                                ssum[:], sums[:, :L], axis=mybir.AxisListType.X,
                                op=ALU.add,
                            )
                            nc.vector.reciprocal(rsum[:], ssum[:])
                        Ps[k] = p_t
                        Rs[k] = rsum

                    # ---- upfront transpose sweep: P^T in [keys, queries] ----
                    # two key tiles (4 transposes) per PSUM tile, one copy each
                    pts = []
                    for j in range(nt // 2):
                        tp_ps = tpp.tile([P, 512], BF16, name="tp", tag="tp")
                        for half in range(2):
                            t_ = 2 * j + half
                            nc.tensor.transpose(
                                tp_ps[:, 256 * half:256 * half + P],
                                Ps[k0][:, P * t_:P * (t_ + 1)], ident_sb[:],
                            )
                            nc.tensor.transpose(
                                tp_ps[:, 256 * half + P:256 * half + 256],
                                Ps[k1][:, P * t_:P * (t_ + 1)], ident_sb[:],
                            )
                        pt_t = ptpool.tile([P, 512], BF16, name="pt", tag="pt")
                        nc.vector.tensor_copy(pt_t[:], tp_ps[:])
                        pts.append(pt_t)

                    # ---- Z = x^T P^T, accumulated over the pair's key range ----
                    zps = [
                        opp.tile([P, 512], FP32, name=f"z{g}_{j}", tag="op")
                        for j in range(4)
                    ]
                    for t_ in range(nt):
                        rhs = pts[t_ // 2][:, 256 * (t_ % 2):256 * (t_ % 2) + 256]
                        for d in range(DC):
                            # one accumulation group per PSUM bank: start
                            # only on the bank's first matmul (whole-bank
                            # pending-zero makes the sibling column-half's
                            # first write an overwrite), stop on its last
                            nc.tensor.matmul(
                                zps[d // 2][:, 256 * (d % 2):256 * (d % 2) + 256],
                                xN[t_][:, P * d:P * (d + 1)],
                                rhs,
                                start=(t_ == 0 and d % 2 == 0),
                                stop=(t_ == nt - 1 and d % 2 == 1),
                            )
                    Z = []
                    for d in range(DC):
                        zt = zpool.tile([P, 256], BF16, name=f"z{g}_{d}", tag="z")
                        src = zps[d // 2][:, 256 * (d % 2):256 * (d % 2) + 256]
                        if d % 2 == 0:
                            nc.vector.tensor_copy(zt[:], src)
                        else:
                            nc.scalar.activation(zt[:], src, AF.Copy)
                        Z.append(zt)

                    # ---- O^T = Wv^T Z ----
                    oT = []
                    for ip in range(4):
                        ps = opp.tile([P, 512], FP32, name=f"ot_ps{g}_{ip}",
                                      tag="op")
                        for d in range(DC):
                            for half in range(2):
                                i = 2 * ip + half
                                nc.tensor.matmul(
                                    ps[:, 256 * half:256 * half + 256],
                                    wv[d][:, P * i:P * (i + 1)],
                                    Z[d][:],
                                    start=(d == 0 and half == 0),
                                    stop=(d == DC - 1 and half == 1),
                                )
                        for half in range(2):
                            i = 2 * ip + half
                            ot = otpool.tile([P, 256], BF16, name=f"ot{g}_{i}",
                                             tag="ot")
                            src = ps[:, 256 * half:256 * half + 256]
                            if half == 0:
                                nc.vector.tensor_copy(ot[:], src)
                            else:
                                nc.scalar.activation(ot[:], src, AF.Copy)
                            oT.append(ot)

                    # ---- output projection for this group's 2 subtiles ----
                    # y = (oT.T @ wo) * (1/rowsum) + b; the rowsum scale rides
                    # the scalar-engine PSUM drain (queries are on partitions)
                    for col, k in enumerate((k0, k1)):
                        for oh in range(2):
                            ps = opp.tile([P, 512], FP32, name="ps_y", tag="op")
                            for i in range(DC):
                                nc.tensor.matmul(
                                    ps[:],
                                    oT[i][:, P * col:P * (col + 1)],
                                    wo[i][:, 512 * oh:512 * (oh + 1)],
                                    start=(i == 0), stop=(i == DC - 1),
                                )
                            y_sb = ypool.tile([P, 512], FP32, name="y_sb", tag="y")
                            # the very last drain runs in two halves so the
                            # scale/bias/DMA chain pipelines at kernel end
                            halves = 2 if (g == 3 and col == 1 and oh == 1) else 1
                            hw_ = 512 // halves
                            for hh in range(halves):
                                sl = slice(hw_ * hh, hw_ * (hh + 1))
                                nc.scalar.activation(
                                    y_sb[:, sl], ps[:, sl], AF.Copy, scale=Rs[k][:]
                                )
                                nc.vector.tensor_tensor(
                                    out=y_sb[:, sl], in0=y_sb[:, sl],
                                    in1=b_sb[:, 512 * oh + hw_ * hh:
                                             512 * oh + hw_ * (hh + 1)],
                                    op=ALU.add,
                                )
                                nc.sync.dma_start(
                                    out=y[P * k:P * (k + 1),
                                          512 * oh + hw_ * hh:
                                          512 * oh + hw_ * (hh + 1)],
                                    in_=y_sb[:, sl],
                                )

    nc.compile()
    return nc


def _prep_inputs(x, w_qkv, w_out, b_out):
    import ml_dtypes
    bf = ml_dtypes.bfloat16
    x = np.asarray(x, dtype=np.float32)
    w_qkv = np.asarray(w_qkv, dtype=np.float32)

    wq = w_qkv[:, 0 * NI:1 * NI]
    wk = w_qkv[:, 1 * NI:2 * NI]
    m = np.ascontiguousarray((wq @ wk.T).astype(bf))
    wv = np.ascontiguousarray(w_qkv[:, 2 * NI:3 * NI].astype(bf))
    wo = np.ascontiguousarray(np.asarray(w_out, dtype=np.float32).astype(bf))
    b_bcast = np.ascontiguousarray(
        np.broadcast_to(np.asarray(b_out, dtype=np.float32)[None, :], (P, NO))
    ).astype(bf)
    ident = np.eye(P, dtype=np.float32).astype(bf)

    xNs = [np.ascontiguousarray(x[b].astype(bf)) for b in range(B)]
    xTs = [np.ascontiguousarray(xNs[b].T) for b in range(B)]

    in_maps = []
    cpos = np.arange(512)[None, :]
    prow = np.arange(P)[:, None]
    for c in range(NCORES):
        b, h = c // 2, c % 2
        subs = [2 * k + h for k in range(NSUB)]
        xQ = np.ascontiguousarray(np.concatenate(
            [xTs[b][:, P * s:P * (s + 1)] for s in subs], axis=1
        ))
        # two distinct diagonal masks: even local subtiles sit at chunk
        # offset 128h, odd ones at 256 + 128h
        mk = np.empty((P, 2 * 512), dtype=bf)
        for par in range(2):
            off = 128 * h + 256 * par
            mk[:, 512 * par:512 * (par + 1)] = np.where(
                cpos <= off + prow, 0.0, NEG
            )
        in_maps.append({
            "xT": xTs[b], "xN": xNs[b], "xQ": xQ,
            "m": m, "wv": wv, "wo": wo,
            "masks": mk, "bb": b_bcast, "ident": ident,
        })
    return in_maps


def _run(x, w_qkv, w_out, b_out, trace=False, **kw):
    if "nc" not in _CACHED:
        _CACHED["nc"] = _build()
    nc = _CACHED["nc"]
    in_maps = _prep_inputs(x, w_qkv, w_out, b_out)
    res = run_bass_kernel_spmd(nc, in_maps, list(range(NCORES)), trace=trace, **kw)
    out = np.empty((B, S, NO), dtype=np.float32)
    for c in range(NCORES):
        b, h = c // 2, c % 2
        yc = res.results[c]["y"]
        for k in range(NSUB):
            s = 2 * k + h
            out[b, P * s:P * (s + 1), :] = yc[P * k:P * (k + 1), :]
    return out, res


def kernel(x, w_qkv, w_out, b_out):
    out, _ = _run(x, w_qkv, w_out, b_out, trace=False)
    return out


# revision 18
# speedup vs baseline: 2.7777x; 1.3045x over previous
"""Causal single-head attention block for Trainium2, SPMD across 8 NeuronCores.

Problem (hardcoded):
    x:     [4, 2048, 1024] f32
    w_qkv: [1024, 3072]    f32   (q | k | v column blocks)
    w_out: [1024, 1024]    f32
    b_out: [1024]          f32
    y = softmax(causal(q @ k.T / 32)) @ v @ w_out + b_out     -> [4, 2048, 1024]

Sharding: 2 cores per batch element. Within a batch, the 16 query subtiles of
128 rows are dealt round-robin to the core pair (core parity h gets subtiles
s = 2k + h, k = 0..7) so both cores see the identical causal work profile
(key-chunk counts [1,1,2,2,3,3,4,4]) and a single SPMD program serves all 8
cores; per-core behavior differs only through input data.

Algebraic restructure (kills K/V production and the output projection's
first factor entirely):
    sim = (xQ^T Wq)(Wk^T x^T) = xQ^T M x^T,   M = Wq Wk^T  (host-precomputed)
        -> AT = M^T xQ on-device (cost of the old Q^T pass), then sim runs
           directly against the resident x^T tiles.
    y   = P (x Wv) Wo = (x^T P^T)^T W2,       W2 = Wv Wo   (host-precomputed)
        -> Z = x^T P^T accumulates against natural-layout x tiles (cost of
           the old attn@V pass), then y = Z^T W2 is the only projection.

All matmul operands are bf16 (SBUF); accumulation is f32 in PSUM. bf16
weights take the fast-weight-load path so LDWEIGHTS hides under the matmuls.
Softmax normalization is postponed: unnormalized exp(sim) feeds Z and the
1/rowsum rides the output-projection PSUM drain (queries on partitions).
"""

import numpy as np

import concourse.mybir as mybir
import concourse.tile as tile
from concourse import bacc
from concourse.bass_utils import run_bass_kernel_spmd

FP32 = mybir.dt.float32
BF16 = mybir.dt.bfloat16
AF = mybir.ActivationFunctionType
ALU = mybir.AluOpType

B, S, D, NI, NO = 4, 2048, 1024, 1024, 1024
NCORES = 8
P = 128
DC = D // P    # 8 contraction chunks over the model dim
RC = S // 512  # 4 key chunks of 512
NT = S // P    # 16 key tiles of 128
NSUB = 8       # local 128-row query subtiles per core
CC = [k // 2 + 1 for k in range(NSUB)]  # 512-key chunks per local subtile
SCALE = float(NI) ** -0.5
NEG = -1.0e9

_CACHED = {}


def _build():
    nc = bacc.Bacc(None, target_bir_lowering=False, debug=False, num_devices=NCORES)

    xT = nc.dram_tensor("xT", [D, S], BF16, kind="ExternalInput").ap()
    xN_d = nc.dram_tensor("xN", [S, D], BF16, kind="ExternalInput").ap()
    xQ = nc.dram_tensor("xQ", [D, NSUB * P], BF16, kind="ExternalInput").ap()
    m_d = nc.dram_tensor("m", [D, D], BF16, kind="ExternalInput").ap()
    w2_d = nc.dram_tensor("w2", [D, NO], BF16, kind="ExternalInput").ap()
    masks = nc.dram_tensor("masks", [P, 2 * 512], BF16, kind="ExternalInput").ap()
    bb = nc.dram_tensor("bb", [P, NO], BF16, kind="ExternalInput").ap()
    ident = nc.dram_tensor("ident", [P, P], BF16, kind="ExternalInput").ap()
    y = nc.dram_tensor("y", [NSUB * P, NO], FP32, kind="ExternalOutput").ap()

    with tile.TileContext(nc) as tc:
        with (
            tc.tile_pool(name="const", bufs=1) as constp,
            tc.tile_pool(name="atpool", bufs=DC) as atp,
            tc.tile_pool(name="xfp", bufs=RC * DC) as xfp,
            tc.tile_pool(name="xnp", bufs=NT) as xnp,
            tc.tile_pool(name="w2pool", bufs=DC) as w2p,
            tc.tile_pool(name="accp", bufs=2, space="PSUM") as accp,
        ):
            AT = [atp.tile([P, NSUB * P], BF16, name=f"at{i}", tag="at")
                  for i in range(DC)]

            with (
                tc.tile_pool(name="mp", bufs=DC) as mp,
                tc.tile_pool(name="xqp", bufs=DC) as xqp,
            ):
                # ---- all input DMAs, emitted in consumption priority order ----
                m_t = []
                xq = []
                for d in range(DC):
                    t = mp.tile([P, D], BF16, name=f"m{d}", tag="m")
                    nc.sync.dma_start(out=t[:], in_=m_d[P * d:P * (d + 1), :])
                    m_t.append(t)
                    t = xqp.tile([P, NSUB * P], BF16, name=f"xq{d}", tag="xq")
                    nc.sync.dma_start(out=t[:], in_=xQ[P * d:P * (d + 1), :])
                    xq.append(t)
                ident_sb = constp.tile([P, P], BF16, name="ident_sb", tag="id")
                nc.sync.dma_start(out=ident_sb[:], in_=ident[:])
                b_sb = constp.tile([P, NO], BF16, name="b_sb", tag="b")
                nc.sync.dma_start(out=b_sb[:], in_=bb[:])
                mask_sb = constp.tile([P, 2 * 512], BF16, name="mask_sb", tag="mask")
                nc.sync.dma_start(out=mask_sb[:], in_=masks[:])
                # remaining inputs interleaved in the order the attention
                # groups consume them, so group 0 never waits on group 3's x
                xfc = [[None] * DC for _ in range(RC)]
                xN = [None] * NT
                w2 = []

                def load_xfc(rc):
                    for d in range(DC):
                        t = xfp.tile([P, 512], BF16, name=f"xf{rc}_{d}", tag="xf")
                        nc.sync.dma_start(
                            out=t[:],
                            in_=xT[P * d:P * (d + 1), 512 * rc:512 * (rc + 1)],
                        )
                        xfc[rc][d] = t

                def load_xn(lo, hi):
                    for t_ in range(lo, hi):
                        t = xnp.tile([P, D], BF16, name=f"xn{t_}", tag="xn")
                        nc.sync.dma_start(
                            out=t[:], in_=xN_d[P * t_:P * (t_ + 1), :]
                        )
                        xN[t_] = t

                load_xfc(0)
                load_xn(0, 4)
                for d in range(DC):
                    t = w2p.tile([P, NO], BF16, name=f"w2{d}", tag="w2")
                    nc.sync.dma_start(out=t[:], in_=w2_d[P * d:P * (d + 1), :])
                    w2.append(t)
                load_xfc(1)
                load_xn(4, 8)
                load_xfc(2)
                load_xn(8, 12)
                load_xfc(3)
                load_xn(12, 16)

                # ---- Phase 1: AT = M^T xQ for the local 1024 queries ----
                for qh in range(2):
                    for i in range(DC):
                        ps = accp.tile([P, 512], FP32, name="ps_at", tag="acc")
                        for d in range(DC):
                            nc.tensor.matmul(
                                ps[:], m_t[d][:, P * i:P * (i + 1)],
                                xq[d][:, 512 * qh:512 * (qh + 1)],
                                start=(d == 0), stop=(d == DC - 1),
                            )
                        if i % 2 == 0:
                            nc.vector.tensor_copy(
                                AT[i][:, 512 * qh:512 * (qh + 1)], ps[:]
                            )
                        else:
                            nc.scalar.activation(
                                AT[i][:, 512 * qh:512 * (qh + 1)], ps[:], AF.Copy
                            )

            # ---- attention, 4 pair-groups of 2 subtiles ----
            with (
                tc.tile_pool(name="ppool", bufs=3) as ppool,
                tc.tile_pool(name="ptpool", bufs=8) as ptpool,
                tc.tile_pool(name="zpool", bufs=8) as zpool,
                tc.tile_pool(name="ypool", bufs=4) as ypool,
                tc.tile_pool(name="stp", bufs=12) as stp,
                tc.tile_pool(name="tpp", bufs=2, space="PSUM") as tpp,
                tc.tile_pool(name="opp", bufs=4, space="PSUM") as opp,
            ):
                for g in range(4):
                    L = g + 1
                    k0, k1 = 2 * g, 2 * g + 1
                    nt = 4 * L
                    Ps = {}
                    Rs = {}
                    for k in (k0, k1):
                        p_t = ppool.tile([P, 4 * 512], BF16, name=f"p{k}", tag="p")
                        sums = stp.tile([P, 4], FP32, name=f"sums{k}", tag="sums")
                        # ascending chunk order (diagonal last) lines up with
                        # the upfront transpose sweep that follows
                        for kc in range(L):
                            diag = kc == L - 1
                            # even subtiles only attend 256 into their
                            # diagonal chunk; the rest is masked anyway
                            w = 256 if (diag and k % 2 == 0) else 512
                            ps = accp.tile([P, 512], FP32, name="ps_sim", tag="acc")
                            for d in range(DC):
                                nc.tensor.matmul(
                                    ps[:, :w],
                                    AT[d][:, P * k:P * (k + 1)],
                                    xfc[kc][d][:, :w],
                                    start=(d == 0), stop=(d == DC - 1),
                                )
                            if diag:
                                mo = 512 * (k % 2)
                                nc.vector.tensor_tensor(
                                    out=ps[:, :w], in0=ps[:, :w],
                                    in1=mask_sb[:, mo:mo + w], op=ALU.add,
                                )
                            nc.scalar.activation(
                                p_t[:, 512 * kc:512 * kc + w], ps[:, :w], AF.Exp,
                                scale=SCALE, accum_out=sums[:, kc:kc + 1],
                            )
                            if w == 256:
                                nc.gpsimd.memset(
                                    p_t[:, 512 * kc + 256:512 * (kc + 1)], 0.0
                                )
                        rsum = stp.tile([P, 1], FP32, name=f"rsum{k}", tag="rs")
                        if L == 1:
                            nc.vector.reciprocal(rsum[:], sums[:, 0:1])
                        else:
                            ssum = stp.tile([P, 1], FP32, name=f"ssum{k}", tag="ss")
                            nc.vector.tensor_reduce(
                                ssum[:], sums[:, :L], axis=mybir.AxisListType.X,
                                op=ALU.add,
                            )
                            nc.vector.reciprocal(rsum[:], ssum[:])
                        Ps[k] = p_t
                        Rs[k] = rsum

                    # ---- upfront transpose sweep: P^T in [keys, queries] ----
                    # two key tiles (4 transposes) per PSUM tile, one copy each
                    pts = []
                    for j in range(nt // 2):
                        tp_ps = tpp.tile([P, 512], BF16, name="tp", tag="tp")
                        for half in range(2):
                            t_ = 2 * j + half
                            nc.tensor.transpose(
                                tp_ps[:, 256 * half:256 * half + P],
                                Ps[k0][:, P * t_:P * (t_ + 1)], ident_sb[:],
                            )
                            nc.tensor.transpose(
                                tp_ps[:, 256 * half + P:256 * half + 256],
                                Ps[k1][:, P * t_:P * (t_ + 1)], ident_sb[:],
                            )
                        pt_t = ptpool.tile([P, 512], BF16, name="pt", tag="pt")
                        nc.vector.tensor_copy(pt_t[:], tp_ps[:])
                        pts.append(pt_t)

                    # ---- Z = x^T P^T, accumulated over the pair's key range ----
                    zps = [
                        opp.tile([P, 512], FP32, name=f"z{g}_{j}", tag="op")
                        for j in range(4)
                    ]
                    for t_ in range(nt):
                        rhs = pts[t_ // 2][:, 256 * (t_ % 2):256 * (t_ % 2) + 256]
                        for d in range(DC):
                            # one accumulation group per PSUM bank: start
                            # only on the bank's first matmul (whole-bank
                            # pending-zero makes the sibling column-half's
                            # first write an overwrite), stop on its last
                            nc.tensor.matmul(
                                zps[d // 2][:, 256 * (d % 2):256 * (d % 2) + 256],
                                xN[t_][:, P * d:P * (d + 1)],
                                rhs,
                                start=(t_ == 0 and d % 2 == 0),
                                stop=(t_ == nt - 1 and d % 2 == 1),
                            )
                    Z = []
                    for d in range(DC):
                        zt = zpool.tile([P, 256], BF16, name=f"z{g}_{d}", tag="z")
                        src = zps[d // 2][:, 256 * (d % 2):256 * (d % 2) + 256]
                        if d % 2 == 0:
                            nc.vector.tensor_copy(zt[:], src)
                        else:
                            nc.scalar.activation(zt[:], src, AF.Copy)
                        Z.append(zt)

                    # ---- output projection: y = Z^T W2 (W2 = Wv Wo, host) ----
                    # the rowsum scale rides the scalar-engine PSUM drain
                    # (queries are on partitions there)
                    for col, k in enumerate((k0, k1)):
                        for oh in range(2):
                            ps = opp.tile([P, 512], FP32, name="ps_y", tag="op")
                            for d in range(DC):
                                nc.tensor.matmul(
                                    ps[:],
                                    Z[d][:, P * col:P * (col + 1)],
                                    w2[d][:, 512 * oh:512 * (oh + 1)],
                                    start=(d == 0), stop=(d == DC - 1),
                                )
                            y_sb = ypool.tile([P, 512], FP32, name="y_sb", tag="y")
                            # the very last drain runs in two halves so the
                            # scale/bias/DMA chain pipelines at kernel end
                            halves = 2 if (g == 3 and col == 1 and oh == 1) else 1
                            hw_ = 512 // halves
                            for hh in range(halves):
                                sl = slice(hw_ * hh, hw_ * (hh + 1))
                                nc.scalar.activation(
                                    y_sb[:, sl], ps[:, sl], AF.Copy, scale=Rs[k][:]
                                )
                                nc.vector.tensor_tensor(
                                    out=y_sb[:, sl], in0=y_sb[:, sl],
                                    in1=b_sb[:, 512 * oh + hw_ * hh:
                                             512 * oh + hw_ * (hh + 1)],
                                    op=ALU.add,
                                )
                                nc.sync.dma_start(
                                    out=y[P * k:P * (k + 1),
                                          512 * oh + hw_ * hh:
                                          512 * oh + hw_ * (hh + 1)],
                                    in_=y_sb[:, sl],
                                )

    nc.compile()
    return nc


def _prep_inputs(x, w_qkv, w_out, b_out):
    import ml_dtypes
    bf = ml_dtypes.bfloat16
    x = np.asarray(x, dtype=np.float32)
    w_qkv = np.asarray(w_qkv, dtype=np.float32)

    wq = w_qkv[:, 0 * NI:1 * NI]
    wk = w_qkv[:, 1 * NI:2 * NI]
    wv = w_qkv[:, 2 * NI:3 * NI]
    m = np.ascontiguousarray((wq @ wk.T).astype(bf))
    w2 = np.ascontiguousarray(
        (wv @ np.asarray(w_out, dtype=np.float32)).astype(bf)
    )
    b_bcast = np.ascontiguousarray(
        np.broadcast_to(np.asarray(b_out, dtype=np.float32)[None, :], (P, NO))
    ).astype(bf)
    ident = np.eye(P, dtype=np.float32).astype(bf)

    xNs = [np.ascontiguousarray(x[b].astype(bf)) for b in range(B)]
    xTs = [np.ascontiguousarray(xNs[b].T) for b in range(B)]

    in_maps = []
    cpos = np.arange(512)[None, :]
    prow = np.arange(P)[:, None]
    for c in range(NCORES):
        b, h = c // 2, c % 2
        subs = [2 * k + h for k in range(NSUB)]
        xQ = np.ascontiguousarray(np.concatenate(
            [xTs[b][:, P * s:P * (s + 1)] for s in subs], axis=1
        ))
        # two distinct diagonal masks: even local subtiles sit at chunk
        # offset 128h, odd ones at 256 + 128h
        mk = np.empty((P, 2 * 512), dtype=bf)
        for par in range(2):
            off = 128 * h + 256 * par
            mk[:, 512 * par:512 * (par + 1)] = np.where(
                cpos <= off + prow, 0.0, NEG
            )
        in_maps.append({
            "xT": xTs[b], "xN": xNs[b], "xQ": xQ,
            "m": m, "w2": w2,
            "masks": mk, "bb": b_bcast, "ident": ident,
        })
    return in_maps


def _run(x, w_qkv, w_out, b_out, trace=False, **kw):
    if "nc" not in _CACHED:
        _CACHED["nc"] = _build()
    nc = _CACHED["nc"]
    in_maps = _prep_inputs(x, w_qkv, w_out, b_out)
    res = run_bass_kernel_spmd(nc, in_maps, list(range(NCORES)), trace=trace, **kw)
    out = np.empty((B, S, NO), dtype=np.float32)
    for c in range(NCORES):
        b, h = c // 2, c % 2
        yc = res.results[c]["y"]
        for k in range(NSUB):
            s = 2 * k + h
            out[b, P * s:P * (s + 1), :] = yc[P * k:P * (k + 1), :]
    return out, res


def kernel(x, w_qkv, w_out, b_out):
    out, _ = _run(x, w_qkv, w_out, b_out, trace=False)
    return out


# revision 19
# speedup vs baseline: 2.7840x; 1.0023x over previous
"""Causal single-head attention block for Trainium2, SPMD across 8 NeuronCores.

Problem (hardcoded):
    x:     [4, 2048, 1024] f32
    w_qkv: [1024, 3072]    f32   (q | k | v column blocks)
    w_out: [1024, 1024]    f32
    b_out: [1024]          f32
    y = softmax(causal(q @ k.T / 32)) @ v @ w_out + b_out     -> [4, 2048, 1024]

Sharding: 2 cores per batch element. Within a batch, the 16 query subtiles of
128 rows are dealt round-robin to the core pair (core parity h gets subtiles
s = 2k + h, k = 0..7) so both cores see the identical causal work profile
(key-chunk counts [1,1,2,2,3,3,4,4]) and a single SPMD program serves all 8
cores; per-core behavior differs only through input data.

Algebraic restructure (kills K/V production and the output projection's
first factor entirely):
    sim = (xQ^T Wq)(Wk^T x^T) = xQ^T M x^T,   M = Wq Wk^T  (host-precomputed)
        -> AT = M^T xQ on-device (cost of the old Q^T pass), then sim runs
           directly against the resident x^T tiles.
    y   = P (x Wv) Wo = (x^T P^T)^T W2,       W2 = Wv Wo   (host-precomputed)
        -> Z = x^T P^T accumulates against natural-layout x tiles (cost of
           the old attn@V pass), then y = Z^T W2 is the only projection.

All matmul operands are bf16 (SBUF); accumulation is f32 in PSUM. bf16
weights take the fast-weight-load path so LDWEIGHTS hides under the matmuls.
Softmax normalization is postponed: unnormalized exp(sim) feeds Z and the
1/rowsum rides the output-projection PSUM drain (queries on partitions).
"""

import numpy as np

import concourse.mybir as mybir
import concourse.tile as tile
from concourse import bacc
from concourse.bass_utils import run_bass_kernel_spmd

FP32 = mybir.dt.float32
BF16 = mybir.dt.bfloat16
AF = mybir.ActivationFunctionType
ALU = mybir.AluOpType

B, S, D, NI, NO = 4, 2048, 1024, 1024, 1024
NCORES = 8
P = 128
DC = D // P    # 8 contraction chunks over the model dim
RC = S // 512  # 4 key chunks of 512
NT = S // P    # 16 key tiles of 128
NSUB = 8       # local 128-row query subtiles per core
CC = [k // 2 + 1 for k in range(NSUB)]  # 512-key chunks per local subtile
SCALE = float(NI) ** -0.5
NEG = -1.0e9

_CACHED = {}


def _build():
    nc = bacc.Bacc(None, target_bir_lowering=False, debug=False, num_devices=NCORES)

    xT = nc.dram_tensor("xT", [D, S], BF16, kind="ExternalInput").ap()
    xN_d = nc.dram_tensor("xN", [S, D], BF16, kind="ExternalInput").ap()
    xQ = nc.dram_tensor("xQ", [D, NSUB * P], BF16, kind="ExternalInput").ap()
    m_d = nc.dram_tensor("m", [D, D], BF16, kind="ExternalInput").ap()
    w2_d = nc.dram_tensor("w2", [D, NO], BF16, kind="ExternalInput").ap()
    masks = nc.dram_tensor("masks", [P, 2 * 512], BF16, kind="ExternalInput").ap()
    bb = nc.dram_tensor("bb", [P, NO], BF16, kind="ExternalInput").ap()
    ident = nc.dram_tensor("ident", [P, P], BF16, kind="ExternalInput").ap()
    y = nc.dram_tensor("y", [NSUB * P, NO], FP32, kind="ExternalOutput").ap()

    with tile.TileContext(nc) as tc:
        with (
            tc.tile_pool(name="const", bufs=1) as constp,
            tc.tile_pool(name="atpool", bufs=DC) as atp,
            tc.tile_pool(name="xfp", bufs=RC * DC) as xfp,
            tc.tile_pool(name="xnp", bufs=NT) as xnp,
            tc.tile_pool(name="w2pool", bufs=DC) as w2p,
            tc.tile_pool(name="accp", bufs=2, space="PSUM") as accp,
        ):
            AT = [atp.tile([P, NSUB * P], BF16, name=f"at{i}", tag="at")
                  for i in range(DC)]

            with (
                tc.tile_pool(name="mp", bufs=DC) as mp,
                tc.tile_pool(name="xqp", bufs=DC) as xqp,
            ):
                # ---- all input DMAs, emitted in consumption priority order ----
                # m/xq split into column halves so the first AT psum is
                # gated on 2MB of arrivals instead of 4MB
                m_t = [mp.tile([P, D], BF16, name=f"m{d}", tag="m")
                       for d in range(DC)]
                xq = [xqp.tile([P, NSUB * P], BF16, name=f"xq{d}", tag="xq")
                      for d in range(DC)]
                for h2 in range(2):
                    for d in range(DC):
                        nc.sync.dma_start(
                            out=m_t[d][:, 512 * h2:512 * (h2 + 1)],
                            in_=m_d[P * d:P * (d + 1), 512 * h2:512 * (h2 + 1)],
                        )
                        nc.sync.dma_start(
                            out=xq[d][:, 512 * h2:512 * (h2 + 1)],
                            in_=xQ[P * d:P * (d + 1), 512 * h2:512 * (h2 + 1)],
                        )
                ident_sb = constp.tile([P, P], BF16, name="ident_sb", tag="id")
                nc.sync.dma_start(out=ident_sb[:], in_=ident[:])
                b_sb = constp.tile([P, NO], BF16, name="b_sb", tag="b")
                nc.sync.dma_start(out=b_sb[:], in_=bb[:])
                mask_sb = constp.tile([P, 2 * 512], BF16, name="mask_sb", tag="mask")
                nc.sync.dma_start(out=mask_sb[:], in_=masks[:])
                # remaining inputs interleaved in the order the attention
                # groups consume them, so group 0 never waits on group 3's x
                xfc = [[None] * DC for _ in range(RC)]
                xN = [None] * NT
                w2 = []

                def load_xfc(rc):
                    for d in range(DC):
                        t = xfp.tile([P, 512], BF16, name=f"xf{rc}_{d}", tag="xf")
                        nc.sync.dma_start(
                            out=t[:],
                            in_=xT[P * d:P * (d + 1), 512 * rc:512 * (rc + 1)],
                        )
                        xfc[rc][d] = t

                def load_xn(lo, hi):
                    for t_ in range(lo, hi):
                        t = xnp.tile([P, D], BF16, name=f"xn{t_}", tag="xn")
                        nc.sync.dma_start(
                            out=t[:], in_=xN_d[P * t_:P * (t_ + 1), :]
                        )
                        xN[t_] = t

                load_xfc(0)
                load_xn(0, 4)
                for d in range(DC):
                    t = w2p.tile([P, NO], BF16, name=f"w2{d}", tag="w2")
                    nc.sync.dma_start(out=t[:], in_=w2_d[P * d:P * (d + 1), :])
                    w2.append(t)
                load_xfc(1)
                load_xn(4, 8)
                load_xfc(2)
                load_xn(8, 12)
                load_xfc(3)
                load_xn(12, 16)

                # ---- Phase 1: AT = M^T xQ for the local 1024 queries ----
                for qh in range(2):
                    for i in range(DC):
                        ps = accp.tile([P, 512], FP32, name="ps_at", tag="acc")
                        for d in range(DC):
                            nc.tensor.matmul(
                                ps[:], m_t[d][:, P * i:P * (i + 1)],
                                xq[d][:, 512 * qh:512 * (qh + 1)],
                                start=(d == 0), stop=(d == DC - 1),
                            )
                        if i % 2 == 0:
                            nc.vector.tensor_copy(
                                AT[i][:, 512 * qh:512 * (qh + 1)], ps[:]
                            )
                        else:
                            nc.scalar.activation(
                                AT[i][:, 512 * qh:512 * (qh + 1)], ps[:], AF.Copy
                            )

            # ---- attention, 4 pair-groups of 2 subtiles ----
            with (
                tc.tile_pool(name="ppool", bufs=3) as ppool,
                tc.tile_pool(name="ptpool", bufs=8) as ptpool,
                tc.tile_pool(name="zpool", bufs=8) as zpool,
                tc.tile_pool(name="ypool", bufs=4) as ypool,
                tc.tile_pool(name="stp", bufs=12) as stp,
                tc.tile_pool(name="tpp", bufs=2, space="PSUM") as tpp,
                tc.tile_pool(name="opp", bufs=4, space="PSUM") as opp,
            ):
                for g in range(4):
                    L = g + 1
                    k0, k1 = 2 * g, 2 * g + 1
                    nt = 4 * L
                    Ps = {}
                    Rs = {}
                    for k in (k0, k1):
                        p_t = ppool.tile([P, 4 * 512], BF16, name=f"p{k}", tag="p")
                        sums = stp.tile([P, 4], FP32, name=f"sums{k}", tag="sums")
                        # ascending chunk order (diagonal last) lines up with
                        # the upfront transpose sweep that follows
                        for kc in range(L):
                            diag = kc == L - 1
                            # even subtiles only attend 256 into their
                            # diagonal chunk; the rest is masked anyway
                            w = 256 if (diag and k % 2 == 0) else 512
                            ps = accp.tile([P, 512], FP32, name="ps_sim", tag="acc")
                            for d in range(DC):
                                nc.tensor.matmul(
                                    ps[:, :w],
                                    AT[d][:, P * k:P * (k + 1)],
                                    xfc[kc][d][:, :w],
                                    start=(d == 0), stop=(d == DC - 1),
                                )
                            if diag:
                                mo = 512 * (k % 2)
                                nc.vector.tensor_tensor(
                                    out=ps[:, :w], in0=ps[:, :w],
                                    in1=mask_sb[:, mo:mo + w], op=ALU.add,
                                )
                            nc.scalar.activation(
                                p_t[:, 512 * kc:512 * kc + w], ps[:, :w], AF.Exp,
                                scale=SCALE, accum_out=sums[:, kc:kc + 1],
                            )
                            if w == 256:
                                nc.gpsimd.memset(
                                    p_t[:, 512 * kc + 256:512 * (kc + 1)], 0.0
                                )
                        rsum = stp.tile([P, 1], FP32, name=f"rsum{k}", tag="rs")
                        if L == 1:
                            nc.vector.reciprocal(rsum[:], sums[:, 0:1])
                        else:
                            ssum = stp.tile([P, 1], FP32, name=f"ssum{k}", tag="ss")
                            nc.vector.tensor_reduce(
                                ssum[:], sums[:, :L], axis=mybir.AxisListType.X,
                                op=ALU.add,
                            )
                            nc.vector.reciprocal(rsum[:], ssum[:])
                        Ps[k] = p_t
                        Rs[k] = rsum

                    # ---- upfront transpose sweep: P^T in [keys, queries] ----
                    # two key tiles (4 transposes) per PSUM tile, one copy each
                    pts = []
                    for j in range(nt // 2):
                        tp_ps = tpp.tile([P, 512], BF16, name="tp", tag="tp")
                        for half in range(2):
                            t_ = 2 * j + half
                            nc.tensor.transpose(
                                tp_ps[:, 256 * half:256 * half + P],
                                Ps[k0][:, P * t_:P * (t_ + 1)], ident_sb[:],
                            )
                            nc.tensor.transpose(
                                tp_ps[:, 256 * half + P:256 * half + 256],
                                Ps[k1][:, P * t_:P * (t_ + 1)], ident_sb[:],
                            )
                        pt_t = ptpool.tile([P, 512], BF16, name="pt", tag="pt")
                        nc.vector.tensor_copy(pt_t[:], tp_ps[:])
                        pts.append(pt_t)

                    # ---- Z = x^T P^T, accumulated over the pair's key range ----
                    zps = [
                        opp.tile([P, 512], FP32, name=f"z{g}_{j}", tag="op")
                        for j in range(4)
                    ]
                    for t_ in range(nt):
                        rhs = pts[t_ // 2][:, 256 * (t_ % 2):256 * (t_ % 2) + 256]
                        for d in range(DC):
                            # one accumulation group per PSUM bank: start
                            # only on the bank's first matmul (whole-bank
                            # pending-zero makes the sibling column-half's
                            # first write an overwrite), stop on its last
                            nc.tensor.matmul(
                                zps[d // 2][:, 256 * (d % 2):256 * (d % 2) + 256],
                                xN[t_][:, P * d:P * (d + 1)],
                                rhs,
                                start=(t_ == 0 and d % 2 == 0),
                                stop=(t_ == nt - 1 and d % 2 == 1),
                            )
                    Z = []
                    for d in range(DC):
                        zt = zpool.tile([P, 256], BF16, name=f"z{g}_{d}", tag="z")
                        src = zps[d // 2][:, 256 * (d % 2):256 * (d % 2) + 256]
                        if d % 2 == 0:
                            nc.vector.tensor_copy(zt[:], src)
                        else:
                            nc.scalar.activation(zt[:], src, AF.Copy)
                        Z.append(zt)

                    # ---- output projection: y = Z^T W2 (W2 = Wv Wo, host) ----
                    # the rowsum scale rides the scalar-engine PSUM drain
                    # (queries are on partitions there)
                    for col, k in enumerate((k0, k1)):
                        for oh in range(2):
                            ps = opp.tile([P, 512], FP32, name="ps_y", tag="op")
                            for d in range(DC):
                                nc.tensor.matmul(
                                    ps[:],
                                    Z[d][:, P * col:P * (col + 1)],
                                    w2[d][:, 512 * oh:512 * (oh + 1)],
                                    start=(d == 0), stop=(d == DC - 1),
                                )
                            y_sb = ypool.tile([P, 512], FP32, name="y_sb", tag="y")
                            # the very last drain runs in two halves so the
                            # scale/bias/DMA chain pipelines at kernel end
                            halves = 2 if (g == 3 and col == 1 and oh == 1) else 1
                            hw_ = 512 // halves
                            for hh in range(halves):
                                sl = slice(hw_ * hh, hw_ * (hh + 1))
                                nc.scalar.activation(
                                    y_sb[:, sl], ps[:, sl], AF.Copy, scale=Rs[k][:]
                                )
                                nc.vector.tensor_tensor(
                                    out=y_sb[:, sl], in0=y_sb[:, sl],
                                    in1=b_sb[:, 512 * oh + hw_ * hh:
                                             512 * oh + hw_ * (hh + 1)],
                                    op=ALU.add,
                                )
                                nc.sync.dma_start(
                                    out=y[P * k:P * (k + 1),
                                          512 * oh + hw_ * hh:
                                          512 * oh + hw_ * (hh + 1)],
                                    in_=y_sb[:, sl],
                                )

    nc.compile()
    return nc


def _prep_inputs(x, w_qkv, w_out, b_out):
    import ml_dtypes
    bf = ml_dtypes.bfloat16
    x = np.asarray(x, dtype=np.float32)
    w_qkv = np.asarray(w_qkv, dtype=np.float32)

    wq = w_qkv[:, 0 * NI:1 * NI]
    wk = w_qkv[:, 1 * NI:2 * NI]
    wv = w_qkv[:, 2 * NI:3 * NI]
    m = np.ascontiguousarray((wq @ wk.T).astype(bf))
    w2 = np.ascontiguousarray(
        (wv @ np.asarray(w_out, dtype=np.float32)).astype(bf)
    )
    b_bcast = np.ascontiguousarray(
        np.broadcast_to(np.asarray(b_out, dtype=np.float32)[None, :], (P, NO))
    ).astype(bf)
    ident = np.eye(P, dtype=np.float32).astype(bf)

    xNs = [np.ascontiguousarray(x[b].astype(bf)) for b in range(B)]
    xTs = [np.ascontiguousarray(xNs[b].T) for b in range(B)]

    in_maps = []
    cpos = np.arange(512)[None, :]
    prow = np.arange(P)[:, None]
    for c in range(NCORES):
        b, h = c // 2, c % 2
        subs = [2 * k + h for k in range(NSUB)]
        xQ = np.ascontiguousarray(np.concatenate(
            [xTs[b][:, P * s:P * (s + 1)] for s in subs], axis=1
        ))
        # two distinct diagonal masks: even local subtiles sit at chunk
        # offset 128h, odd ones at 256 + 128h
        mk = np.empty((P, 2 * 512), dtype=bf)
        for par in range(2):
            off = 128 * h + 256 * par
            mk[:, 512 * par:512 * (par + 1)] = np.where(
                cpos <= off + prow, 0.0, NEG
            )
        in_maps.append({
            "xT": xTs[b], "xN": xNs[b], "xQ": xQ,
            "m": m, "w2": w2,
            "masks": mk, "bb": b_bcast, "ident": ident,
        })
    return in_maps


def _run(x, w_qkv, w_out, b_out, trace=False, **kw):
    if "nc" not in _CACHED:
        _CACHED["nc"] = _build()
    nc = _CACHED["nc"]
    in_maps = _prep_inputs(x, w_qkv, w_out, b_out)
    res = run_bass_kernel_spmd(nc, in_maps, list(range(NCORES)), trace=trace, **kw)
    out = np.empty((B, S, NO), dtype=np.float32)
    for c in range(NCORES):
        b, h = c // 2, c % 2
        yc = res.results[c]["y"]
        for k in range(NSUB):
            s = 2 * k + h
            out[b, P * s:P * (s + 1), :] = yc[P * k:P * (k + 1), :]
    return out, res


def kernel(x, w_qkv, w_out, b_out):
    out, _ = _run(x, w_qkv, w_out, b_out, trace=False)
    return out


# revision 22
# speedup vs baseline: 2.7991x; 1.0054x over previous
"""Causal single-head attention block for Trainium2, SPMD across 8 NeuronCores.

Problem (hardcoded):
    x:     [4, 2048, 1024] f32
    w_qkv: [1024, 3072]    f32   (q | k | v column blocks)
    w_out: [1024, 1024]    f32
    b_out: [1024]          f32
    y = softmax(causal(q @ k.T / 32)) @ v @ w_out + b_out     -> [4, 2048, 1024]

Sharding: 2 cores per batch element. Within a batch, the 16 query subtiles of
128 rows are dealt round-robin to the core pair (core parity h gets subtiles
s = 2k + h, k = 0..7) so both cores see the identical causal work profile
(key-chunk counts [1,1,2,2,3,3,4,4]) and a single SPMD program serves all 8
cores; per-core behavior differs only through input data.

Algebraic restructure (kills K/V production and the output projection's
first factor entirely):
    sim = (xQ^T Wq)(Wk^T x^T) = xQ^T M x^T,   M = Wq Wk^T  (host-precomputed)
        -> AT = M^T xQ on-device (cost of the old Q^T pass), then sim runs
           directly against the resident x^T tiles.
    y   = P (x Wv) Wo = (x^T P^T)^T W2,       W2 = Wv Wo   (host-precomputed)
        -> Z = x^T P^T accumulates against natural-layout x tiles (cost of
           the old attn@V pass), then y = Z^T W2 is the only projection.

All matmul operands are bf16 (SBUF); accumulation is f32 in PSUM. bf16
weights take the fast-weight-load path so LDWEIGHTS hides under the matmuls.
Softmax normalization is postponed: unnormalized exp(sim) feeds Z and the
1/rowsum rides the output-projection PSUM drain (queries on partitions).
"""

import numpy as np

import concourse.mybir as mybir
import concourse.tile as tile
from concourse import bacc
from concourse.bass_utils import run_bass_kernel_spmd

FP32 = mybir.dt.float32
BF16 = mybir.dt.bfloat16
AF = mybir.ActivationFunctionType
ALU = mybir.AluOpType

B, S, D, NI, NO = 4, 2048, 1024, 1024, 1024
NCORES = 8
P = 128
DC = D // P    # 8 contraction chunks over the model dim
RC = S // 512  # 4 key chunks of 512
NT = S // P    # 16 key tiles of 128
NSUB = 8       # local 128-row query subtiles per core
CC = [k // 2 + 1 for k in range(NSUB)]  # 512-key chunks per local subtile
SCALE = float(NI) ** -0.5
NEG = -1.0e9

_CACHED = {}


def _build():
    nc = bacc.Bacc(None, target_bir_lowering=False, debug=False, num_devices=NCORES)

    xT = nc.dram_tensor("xT", [D, S], BF16, kind="ExternalInput").ap()
    xN_d = nc.dram_tensor("xN", [S, D], BF16, kind="ExternalInput").ap()
    xQ = nc.dram_tensor("xQ", [D, NSUB * P], BF16, kind="ExternalInput").ap()
    m_d = nc.dram_tensor("m", [D, D], BF16, kind="ExternalInput").ap()
    w2_d = nc.dram_tensor("w2", [D, NO], BF16, kind="ExternalInput").ap()
    masks = nc.dram_tensor("masks", [P, 2 * 512], BF16, kind="ExternalInput").ap()
    bb = nc.dram_tensor("bb", [P, NO], BF16, kind="ExternalInput").ap()
    ident = nc.dram_tensor("ident", [P, P], BF16, kind="ExternalInput").ap()
    y = nc.dram_tensor("y", [NSUB * P, NO], FP32, kind="ExternalOutput").ap()

    with tile.TileContext(nc) as tc:
        with (
            tc.tile_pool(name="const", bufs=1) as constp,
            tc.tile_pool(name="atpool", bufs=DC) as atp,
            tc.tile_pool(name="xfp", bufs=RC * DC) as xfp,
            tc.tile_pool(name="xnp", bufs=NT) as xnp,
            tc.tile_pool(name="w2pool", bufs=DC) as w2p,
            tc.tile_pool(name="accp", bufs=2, space="PSUM") as accp,
        ):
            AT = [atp.tile([P, NSUB * P], BF16, name=f"at{i}", tag="at")
                  for i in range(DC)]

            with (
                tc.tile_pool(name="mp", bufs=2 * DC) as mp,
                tc.tile_pool(name="xqp", bufs=2 * DC) as xqp,
            ):
                # ---- all input DMAs, emitted in consumption priority order ----
                # m/xq split into separate column-half tiles (dependency
                # tracking is tile-granular) so the first AT psum is gated
                # on 2MB of arrivals instead of 4MB
                m_t = [[None] * DC for _ in range(2)]
                xq = [[None] * DC for _ in range(2)]
                for h2 in range(2):
                    for d in range(DC):
                        t = mp.tile([P, 512], BF16, name=f"m{h2}_{d}", tag="m")
                        nc.sync.dma_start(
                            out=t[:],
                            in_=m_d[P * d:P * (d + 1), 512 * h2:512 * (h2 + 1)],
                        )
                        m_t[h2][d] = t
                        t = xqp.tile([P, 512], BF16, name=f"xq{h2}_{d}",
                                     tag="xq")
                        nc.sync.dma_start(
                            out=t[:],
                            in_=xQ[P * d:P * (d + 1), 512 * h2:512 * (h2 + 1)],
                        )
                        xq[h2][d] = t
                ident_sb = constp.tile([P, P], BF16, name="ident_sb", tag="id")
                nc.sync.dma_start(out=ident_sb[:], in_=ident[:])
                b_sb = constp.tile([P, NO], BF16, name="b_sb", tag="b")
                nc.sync.dma_start(out=b_sb[:], in_=bb[:])
                mask_sb = constp.tile([P, 2 * 512], BF16, name="mask_sb", tag="mask")
                nc.sync.dma_start(out=mask_sb[:], in_=masks[:])
                # remaining inputs interleaved in the order the attention
                # groups consume them, so group 0 never waits on group 3's x
                xfc = [[None] * DC for _ in range(RC)]
                xN = [None] * NT
                w2 = []

                def load_xfc(rc):
                    for d in range(DC):
                        t = xfp.tile([P, 512], BF16, name=f"xf{rc}_{d}", tag="xf")
                        nc.sync.dma_start(
                            out=t[:],
                            in_=xT[P * d:P * (d + 1), 512 * rc:512 * (rc + 1)],
                        )
                        xfc[rc][d] = t

                def load_xn(lo, hi):
                    for t_ in range(lo, hi):
                        t = xnp.tile([P, D], BF16, name=f"xn{t_}", tag="xn")
                        nc.sync.dma_start(
                            out=t[:], in_=xN_d[P * t_:P * (t_ + 1), :]
                        )
                        xN[t_] = t

                load_xfc(0)
                load_xn(0, 4)
                for d in range(DC):
                    t = w2p.tile([P, NO], BF16, name=f"w2{d}", tag="w2")
                    nc.sync.dma_start(out=t[:], in_=w2_d[P * d:P * (d + 1), :])
                    w2.append(t)
                load_xfc(1)
                load_xn(4, 8)
                load_xfc(2)
                load_xn(8, 12)
                load_xfc(3)
                load_xn(12, 16)

                # ---- Phase 1: AT = M^T xQ for the local 1024 queries ----
                for qh in range(2):
                    for i in range(DC):
                        ps = accp.tile([P, 512], FP32, name="ps_at", tag="acc")
                        for d in range(DC):
                            nc.tensor.matmul(
                                ps[:],
                                m_t[i // 4][d][:, P * (i % 4):P * (i % 4 + 1)],
                                xq[qh][d][:],
                                start=(d == 0), stop=(d == DC - 1),
                            )
                        if i % 2 == 0:
                            nc.vector.tensor_copy(
                                AT[i][:, 512 * qh:512 * (qh + 1)], ps[:]
                            )
                        else:
                            nc.scalar.activation(
                                AT[i][:, 512 * qh:512 * (qh + 1)], ps[:], AF.Copy
                            )

            # ---- attention, 4 pair-groups of 2 subtiles ----
            with (
                tc.tile_pool(name="ppool", bufs=3) as ppool,
                tc.tile_pool(name="ptpool", bufs=8) as ptpool,
                tc.tile_pool(name="zpool", bufs=8) as zpool,
                tc.tile_pool(name="ypool", bufs=4) as ypool,
                tc.tile_pool(name="stp", bufs=12) as stp,
                tc.tile_pool(name="tpp", bufs=2, space="PSUM") as tpp,
                tc.tile_pool(name="opp", bufs=4, space="PSUM") as opp,
            ):
                for g in range(4):
                    L = g + 1
                    k0, k1 = 2 * g, 2 * g + 1
                    nt = 4 * L
                    Ps = {}
                    Rs = {}
                    for k in (k0, k1):
                        p_t = ppool.tile([P, 4 * 512], BF16, name=f"p{k}", tag="p")
                        sums = stp.tile([P, 4], FP32, name=f"sums{k}", tag="sums")
                        # ascending chunk order (diagonal last) lines up with
                        # the upfront transpose sweep that follows
                        for kc in range(L):
                            diag = kc == L - 1
                            # even subtiles only attend 256 into their
                            # diagonal chunk; the rest is masked anyway
                            w = 256 if (diag and k % 2 == 0) else 512
                            ps = accp.tile([P, 512], FP32, name="ps_sim", tag="acc")
                            for d in range(DC):
                                nc.tensor.matmul(
                                    ps[:, :w],
                                    AT[d][:, P * k:P * (k + 1)],
                                    xfc[kc][d][:, :w],
                                    start=(d == 0), stop=(d == DC - 1),
                                )
                            if diag:
                                mo = 512 * (k % 2)
                                nc.vector.tensor_tensor(
                                    out=ps[:, :w], in0=ps[:, :w],
                                    in1=mask_sb[:, mo:mo + w], op=ALU.add,
                                )
                            nc.scalar.activation(
                                p_t[:, 512 * kc:512 * kc + w], ps[:, :w], AF.Exp,
                                scale=SCALE, accum_out=sums[:, kc:kc + 1],
                            )
                            if w == 256:
                                nc.gpsimd.memset(
                                    p_t[:, 512 * kc + 256:512 * (kc + 1)], 0.0
                                )
                        rsum = stp.tile([P, 1], FP32, name=f"rsum{k}", tag="rs")
                        if L == 1:
                            nc.vector.reciprocal(rsum[:], sums[:, 0:1])
                        else:
                            ssum = stp.tile([P, 1], FP32, name=f"ssum{k}", tag="ss")
                            nc.vector.tensor_reduce(
                                ssum[:], sums[:, :L], axis=mybir.AxisListType.X,
                                op=ALU.add,
                            )
                            nc.vector.reciprocal(rsum[:], ssum[:])
                        Ps[k] = p_t
                        Rs[k] = rsum

                    # ---- upfront transpose sweep: P^T in [keys, queries] ----
                    # two key tiles (4 transposes) per PSUM tile, one copy each
                    pts = []
                    for j in range(nt // 2):
                        tp_ps = tpp.tile([P, 512], BF16, name="tp", tag="tp")
                        for half in range(2):
                            t_ = 2 * j + half
                            nc.tensor.transpose(
                                tp_ps[:, 256 * half:256 * half + P],
                                Ps[k0][:, P * t_:P * (t_ + 1)], ident_sb[:],
                            )
                            nc.tensor.transpose(
                                tp_ps[:, 256 * half + P:256 * half + 256],
                                Ps[k1][:, P * t_:P * (t_ + 1)], ident_sb[:],
                            )
                        pt_t = ptpool.tile([P, 512], BF16, name="pt", tag="pt")
                        nc.vector.tensor_copy(pt_t[:], tp_ps[:])
                        pts.append(pt_t)

                    # ---- Z = x^T P^T, accumulated over the pair's key range ----
                    zps = [
                        opp.tile([P, 512], FP32, name=f"z{g}_{j}", tag="op")
                        for j in range(4)
                    ]
                    for t_ in range(nt):
                        rhs = pts[t_ // 2][:, 256 * (t_ % 2):256 * (t_ % 2) + 256]
                        for d in range(DC):
                            # one accumulation group per PSUM bank: start
                            # only on the bank's first matmul (whole-bank
                            # pending-zero makes the sibling column-half's
                            # first write an overwrite), stop on its last
                            nc.tensor.matmul(
                                zps[d // 2][:, 256 * (d % 2):256 * (d % 2) + 256],
                                xN[t_][:, P * d:P * (d + 1)],
                                rhs,
                                start=(t_ == 0 and d % 2 == 0),
                                stop=(t_ == nt - 1 and d % 2 == 1),
                            )
                    Z = []
                    for d in range(DC):
                        zt = zpool.tile([P, 256], BF16, name=f"z{g}_{d}", tag="z")
                        src = zps[d // 2][:, 256 * (d % 2):256 * (d % 2) + 256]
                        if d % 2 == 0:
                            nc.vector.tensor_copy(zt[:], src)
                        else:
                            nc.scalar.activation(zt[:], src, AF.Copy)
                        Z.append(zt)

                    # ---- output projection: y = Z^T W2 (W2 = Wv Wo, host) ----
                    # the rowsum scale rides the scalar-engine PSUM drain
                    # (queries are on partitions there)
                    for col, k in enumerate((k0, k1)):
                        for oh in range(2):
                            ps = opp.tile([P, 512], FP32, name="ps_y", tag="op")
                            for d in range(DC):
                                nc.tensor.matmul(
                                    ps[:],
                                    Z[d][:, P * col:P * (col + 1)],
                                    w2[d][:, 512 * oh:512 * (oh + 1)],
                                    start=(d == 0), stop=(d == DC - 1),
                                )
                            y_sb = ypool.tile([P, 512], FP32, name="y_sb", tag="y")
                            # the very last drain runs in two halves so the
                            # scale/bias/DMA chain pipelines at kernel end
                            halves = 2 if (g == 3 and col == 1 and oh == 1) else 1
                            hw_ = 512 // halves
                            for hh in range(halves):
                                sl = slice(hw_ * hh, hw_ * (hh + 1))
                                nc.scalar.activation(
                                    y_sb[:, sl], ps[:, sl], AF.Copy, scale=Rs[k][:]
                                )
                                nc.vector.tensor_tensor(
                                    out=y_sb[:, sl], in0=y_sb[:, sl],
                                    in1=b_sb[:, 512 * oh + hw_ * hh:
                                             512 * oh + hw_ * (hh + 1)],
                                    op=ALU.add,
                                )
                                nc.sync.dma_start(
                                    out=y[P * k:P * (k + 1),
                                          512 * oh + hw_ * hh:
                                          512 * oh + hw_ * (hh + 1)],
                                    in_=y_sb[:, sl],
                                )

    nc.compile()
    return nc


def _prep_inputs(x, w_qkv, w_out, b_out):
    import ml_dtypes
    bf = ml_dtypes.bfloat16
    x = np.asarray(x, dtype=np.float32)
    w_qkv = np.asarray(w_qkv, dtype=np.float32)

    wq = w_qkv[:, 0 * NI:1 * NI]
    wk = w_qkv[:, 1 * NI:2 * NI]
    wv = w_qkv[:, 2 * NI:3 * NI]
    m = np.ascontiguousarray((wq @ wk.T).astype(bf))
    w2 = np.ascontiguousarray(
        (wv @ np.asarray(w_out, dtype=np.float32)).astype(bf)
    )
    b_bcast = np.ascontiguousarray(
        np.broadcast_to(np.asarray(b_out, dtype=np.float32)[None, :], (P, NO))
    ).astype(bf)
    ident = np.eye(P, dtype=np.float32).astype(bf)

    xNs = [np.ascontiguousarray(x[b].astype(bf)) for b in range(B)]
    xTs = [np.ascontiguousarray(xNs[b].T) for b in range(B)]

    in_maps = []
    cpos = np.arange(512)[None, :]
    prow = np.arange(P)[:, None]
    for c in range(NCORES):
        b, h = c // 2, c % 2
        subs = [2 * k + h for k in range(NSUB)]
        xQ = np.ascontiguousarray(np.concatenate(
            [xTs[b][:, P * s:P * (s + 1)] for s in subs], axis=1
        ))
        # two distinct diagonal masks: even local subtiles sit at chunk
        # offset 128h, odd ones at 256 + 128h
        mk = np.empty((P, 2 * 512), dtype=bf)
        for par in range(2):
            off = 128 * h + 256 * par
            mk[:, 512 * par:512 * (par + 1)] = np.where(
                cpos <= off + prow, 0.0, NEG
            )
        in_maps.append({
            "xT": xTs[b], "xN": xNs[b], "xQ": xQ,
            "m": m, "w2": w2,
            "masks": mk, "bb": b_bcast, "ident": ident,
        })
    return in_maps


def _run(x, w_qkv, w_out, b_out, trace=False, **kw):
    if "nc" not in _CACHED:
        _CACHED["nc"] = _build()
    nc = _CACHED["nc"]
    in_maps = _prep_inputs(x, w_qkv, w_out, b_out)
    res = run_bass_kernel_spmd(nc, in_maps, list(range(NCORES)), trace=trace, **kw)
    out = np.empty((B, S, NO), dtype=np.float32)
    for c in range(NCORES):
        b, h = c // 2, c % 2
        yc = res.results[c]["y"]
        for k in range(NSUB):
            s = 2 * k + h
            out[b, P * s:P * (s + 1), :] = yc[P * k:P * (k + 1), :]
    return out, res


def kernel(x, w_qkv, w_out, b_out):
    out, _ = _run(x, w_qkv, w_out, b_out, trace=False)
    return out
